# revision 3
# baseline (speedup 1.0000x reference)
"""FFM (fast-and-forgetful memory) layer on 8 Trainium2 NeuronCores.

Math: per (trace i, ctx j) channel, complex recurrence
    s_t = gamma_ij * s_{t-1} + z_t,   gamma_ij = exp(-|a_i|) * e^{i b_j}
with z_t = gated[t, i] broadcast over j, followed by
    zm = [Re s; Im s] @ W_mix + b_mix   (contraction over 2*64*64 = 8192)
    out = LN(zm * sigmoid(x@W_gout+b)) + (x@W_skip+b) * (1 - sigmoid(...))

Device decomposition (8 cores, trace-sharded; 3 ReduceScatters):
  A0 : every core computes gated ONLY for its own 8 traces over the FULL
       sequence (x^T streamed from DRAM).
  A1 : rotate the complex scan into two real scans R_t = rho*R_{t-1} + w_t
       (w = e^{-i b t} z) via DVE tensor_tensor_scan; rotation back with
       host-precomputed cos/sin(b_j t) bf16 tables. 8 compute slices of
       512 steps, scan state chained via direct initial=prev[:, -1:] APs.
       Engine split tuned to the cost model: scans + most muls on DVE,
       ss/m2(/m4 on even tiles) on gpsimd.
  A2 : zm partial (own 1024 real channels): per compute slice, 4 psum
       tiles of [128, OUT]; b_mix folded in via ACT psum pre-fill (no
       bias matmuls). bf16 stores; 3 grouped ReduceScatter(add) calls
       (cs 0-2 / 3-5 / 6-7) with issue positions tuned so the Pool queue
       never stalls on them.
  B  : gout/skip matmuls precomputed early; after each RS the core
       finishes LayerNorm + mix for its rows, in 4 row blocks laid out
       host-side so none spans a 128-partition boundary.
"""

import numpy as np
from contextlib import ExitStack

import concourse.bacc as bacc
import concourse.bass as bass
import concourse.tile as tile
from concourse import mybir
from concourse.bass_utils import run_bass_kernel_spmd

T, IN, TR, CTX, OUT = 4096, 512, 64, 64, 512
NCORES = 8
TL = T // NCORES        # 512: output rows per core
TPC = TR // NCORES      # 8 traces per core in the scan phase
NT = TPC // 2           # 4 channel tiles (2 traces x 64 ctx = 128 partitions)
KCH = 2 * NT            # 8 zm K-chunks per core (real+imag per tile)
CSL = 512               # compute-slice length (timesteps)
NCS = T // CSL          # 8 compute slices
GRP_CS = [(0, 3), (3, 6), (6, 8)]          # cs-ranges per RS group
SLS = [(lo * CSL, (hi - lo) * CSL) for lo, hi in GRP_CS]
BLS = [L // NCORES for _, L in SLS]        # (192, 192, 128) B rows / group
RS_POS = {0: 4, 1: 8, 2: 8}                # emit RS g before cs k (8 = end)
WAVE = 2                # psum groups per A2 wave
LN_EPS = 1e-6
FP32 = mybir.dt.float32
BF16 = mybir.dt.bfloat16
AOT = mybir.AluOpType
AFT = mybir.ActivationFunctionType

# B row blocks: (list of (group, row-offset-in-group, nrows)) per 128-block,
# laid out so each block sits in one 128-partition chunk of gout/skip/xb.
B_BLOCKS = [
    [(0, 0, 128)],
    [(1, 0, 128)],
    [(2, 0, 128)],
    [(0, 128, 64), (1, 128, 64)],
]

_CACHE: dict = {}


def _free_bcast(col: bass.AP, n: int) -> bass.AP:
    """Broadcast a [P, 1] column along the free dim to [P, n] via stride 0."""
    return bass.AP(tensor=col.tensor, offset=col.offset, ap=[col.ap[0], [0, n]])


def _build_module(reps: int = 1):
    nc = bacc.Bacc(
        "TRN2", target_bir_lowering=False, debug=False, num_devices=NCORES
    )

    def inp(name, shape, dt):
        return nc.dram_tensor(name, list(shape), dt, kind="ExternalInput").ap()

    xT = inp("xT", (IN, T), BF16)                  # full x, transposed
    xbT = inp("xbT", (IN, TL), BF16)               # x^T cols for B rows
    wpg = inp("wpg", (4, 128, 64), BF16)           # own pre @0..8, gin @32..40
    bias_pg = inp("bias_pg", (64, 1), FP32)        # own b_pre | b_gin
    cosb = inp("cosb", (128, T), BF16)             # cos(b_j t), 2x64 rows
    sinb = inp("sinb", (128, T), BF16)
    rho = inp("rho", (128, NT), FP32)              # exp(-|a_i|) per tile col
    init_cs = inp("init_cs", (128, 2 * NT), FP32)  # scan initials per tile
    wmix = inp("wmix", (KCH, 128, OUT), BF16)      # rearranged W_mix rows
    bmix = inp("bmix", (1, OUT), FP32)             # b_mix on core 0, else 0
    wgout = inp("wgout", (4, 128, OUT), BF16)
    wskip = inp("wskip", (4, 128, OUT), BF16)
    bgout = inp("bgout", (1, OUT), BF16)
    bskip = inp("bskip", (1, OUT), BF16)
    ones_row = inp("ones_row", (1, 128), BF16)

    outc = nc.dram_tensor("outc", [TL, OUT], FP32, kind="ExternalOutput").ap()

    groups = [list(range(NCORES))]

    with tile.TileContext(nc) as tc, ExitStack() as ctx:
        const = ctx.enter_context(tc.tile_pool(name="const", bufs=1))
        dram = ctx.enter_context(tc.tile_pool(name="dram", bufs=1, space="DRAM"))

        # ---- resident constants (heavy loads issued on the Pool queue:
        # DMA_SEQ_TIME is 25ns there vs 565+ elsewhere) ------------------
        rho_sb = const.tile([128, NT], FP32)
        nc.sync.dma_start(rho_sb, rho)
        init_sb = const.tile([128, 2 * NT], FP32)
        nc.sync.dma_start(init_sb, init_cs)
        bias_pg_sb = const.tile([64, 1], FP32)
        nc.sync.dma_start(bias_pg_sb, bias_pg)
        ones_sb = const.tile([1, 128], BF16)
        nc.sync.dma_start(ones_sb, ones_row)
        bgout_sb = const.tile([1, OUT], BF16)
        nc.sync.dma_start(bgout_sb, bgout)
        bskip_sb = const.tile([1, OUT], BF16)
        nc.sync.dma_start(bskip_sb, bskip)
        # b_mix broadcast to all 128 partitions (psum pre-fill source)
        bmixb_sb = const.tile([128, OUT], FP32)
        nc.sync.dma_start(
            bmixb_sb,
            bass.AP(tensor=bmix.tensor, offset=0, ap=[[0, 128], [1, OUT]]),
        )
        eps_sb = const.tile([128, 1], FP32)
        nc.vector.memset(eps_sb, LN_EPS)

        cosb_sb = const.tile([128, T], BF16)
        nc.gpsimd.dma_start(cosb_sb, cosb)
        sinb_sb = const.tile([128, T], BF16)
        nc.gpsimd.dma_start(sinb_sb, sinb)
        wpg_sb = const.tile([128, 4, 64], BF16)
        nc.gpsimd.dma_start(
            wpg_sb,
            bass.AP(tensor=wpg.tensor, offset=0,
                    ap=[[64, 128], [128 * 64, 4], [1, 64]]),
        )
        xb_sb = const.tile([128, 4, TL], BF16)
        nc.gpsimd.dma_start(
            xb_sb,
            bass.AP(tensor=xbT.tensor, offset=0,
                    ap=[[TL, 128], [128 * TL, 4], [1, TL]]),
        )
        wgout_sb = const.tile([128, 4, OUT], BF16)
        nc.gpsimd.dma_start(
            wgout_sb,
            bass.AP(tensor=wgout.tensor, offset=0,
                    ap=[[OUT, 128], [128 * OUT, 4], [1, OUT]]),
        )
        wskip_sb = const.tile([128, 4, OUT], BF16)
        nc.gpsimd.dma_start(
            wskip_sb,
            bass.AP(tensor=wskip.tensor, offset=0,
                    ap=[[OUT, 128], [128 * OUT, 4], [1, OUT]]),
        )
        wmix_sb = const.tile([128, KCH, OUT], BF16)
        nc.gpsimd.dma_start(
            wmix_sb,
            bass.AP(tensor=wmix.tensor, offset=0,
                    ap=[[OUT, 128], [128 * OUT, KCH], [1, OUT]]),
        )

        for _rep in range(reps):
            # ---- phase A0: gated for OWN 8 traces over full T -------------
            gbf = const.tile([TPC, T], BF16, tag="gbf")
            g_loc_d = dram.tile([TPC, T], BF16, name="g_loc_d")
            with tc.tile_pool(name="a0", bufs=4) as a0, \
                    tc.tile_pool(name="psa0", bufs=1, space="PSUM") as psum0:
                for tc8 in range(T // TL):
                    xt_t = a0.tile([128, 4, TL], BF16, tag="xt")
                    nc.sync.dma_start(
                        xt_t,
                        bass.AP(tensor=xT.tensor,
                                offset=tc8 * TL,
                                ap=[[T, 128], [128 * T, 4], [1, TL]]),
                    )
                    ps_pg = psum0.tile([64, TL], FP32, tag="pg", bufs=2)
                    for ki in range(4):
                        nc.tensor.matmul(
                            ps_pg,
                            wpg_sb[:, ki, :],
                            xt_t[:, ki, :],
                            start=(ki == 0),
                            stop=(ki == 3),
                        )
                    pre_sb = a0.tile([TPC, TL], FP32, tag="pre")
                    nc.scalar.activation(
                        pre_sb, ps_pg[0:TPC, :], AFT.Identity,
                        bias=bias_pg_sb[0:TPC, :],
                    )
                    sig_sb = a0.tile([TPC, TL], FP32, tag="sig")
                    nc.scalar.activation(
                        sig_sb, ps_pg[32:32 + TPC, :], AFT.Sigmoid,
                        bias=bias_pg_sb[32:32 + TPC, :],
                    )
                    nc.vector.tensor_mul(
                        gbf[:, tc8 * TL:(tc8 + 1) * TL], pre_sb, sig_sb
                    )
                    nc.gpsimd.dma_start(
                        bass.AP(tensor=g_loc_d.tensor,
                                offset=g_loc_d.offset + tc8 * TL,
                                ap=[[T, TPC], [1, TL]]),
                        gbf[:, tc8 * TL:(tc8 + 1) * TL],
                    )

            # ---- early B-prep: gout/skip for this core's B rows -----------
            gout_st = const.tile([128, 4, OUT], BF16, tag="gout_st")
            skip_st = const.tile([128, 4, OUT], BF16, tag="skip_st")
            with tc.tile_pool(name="psb0", bufs=1, space="PSUM") as psb0:
                for kt in range(4):
                    tloc = kt * 128
                    ps_go = psb0.tile([128, OUT], FP32, tag="go", bufs=2,
                                      name="ps_go")
                    for ki in range(4):
                        nc.tensor.matmul(
                            ps_go,
                            xb_sb[:, ki, tloc:tloc + 128],
                            wgout_sb[:, ki, :],
                            start=(ki == 0),
                            stop=False,
                        )
                    nc.tensor.matmul(
                        ps_go, ones_sb, bgout_sb, start=False, stop=True,
                    )
                    nc.scalar.activation(gout_st[:, kt, :], ps_go,
                                         AFT.Sigmoid)
                    ps_sk = psb0.tile([128, OUT], FP32, tag="sk", bufs=2,
                                      name="ps_sk")
                    for ki in range(4):
                        nc.tensor.matmul(
                            ps_sk,
                            xb_sb[:, ki, tloc:tloc + 128],
                            wskip_sb[:, ki, :],
                            start=(ki == 0),
                            stop=False,
                        )
                    nc.tensor.matmul(
                        ps_sk, ones_sb, bskip_sb, start=False, stop=True,
                    )
                    nc.scalar.copy(skip_st[:, kt, :], ps_sk)

            # ---- phases A1 + A2 pipelined over 8 compute slices ----------
            with tc.tile_pool(name="a1", bufs=1) as a1, \
                    tc.tile_pool(name="psa2", bufs=1, space="PSUM") as psum2, \
                    tc.tile_pool(name="pb", bufs=2) as pb:
                zm_d = [dram.tile([SLS[g][1], OUT], BF16, name=f"zmd{g}")
                        for g in range(3)]
                zm_own_d = [dram.tile([BLS[g], OUT], BF16, name=f"zmo{g}")
                            for g in range(3)]

                C_t = [None] * NT     # per-tile C/S tiles for carry chaining
                S_t = [None] * NT

                def emit_rs(grp):
                    nc.gpsimd.collective_compute(
                        "ReduceScatter", AOT.add, replica_groups=groups,
                        ins=[zm_d[grp].opt()], outs=[zm_own_d[grp].opt()],
                    )

                for cs in range(NCS):
                    for grp, pos in RS_POS.items():
                        if pos == cs:
                            emit_rs(grp)
                    grp = next(gi for gi, (lo, hi) in enumerate(GRP_CS)
                               if lo <= cs < hi)
                    hst = cs * CSL
                    sl = slice(hst, hst + CSL)
                    g_rep = [None] * NT
                    # g_rep broadcast loads for all tiles first (Pool queue)
                    for g in range(NT):
                        gr = a1.tile([128, CSL], BF16, tag=f"grep{g}", bufs=2,
                                     name="g_rep")
                        for il in range(2):
                            nc.gpsimd.dma_start(
                                gr[il * CTX:(il + 1) * CTX, :],
                                bass.AP(
                                    tensor=g_loc_d.tensor,
                                    offset=(g_loc_d.offset
                                            + (2 * g + il) * T + hst),
                                    ap=[[0, CTX], [1, CSL]],
                                ),
                            )
                        g_rep[g] = gr
                    # Pool: ss for all tiles (feeds the S scans)
                    ss_t = [None] * NT
                    for g in range(NT):
                        ss = a1.tile([128, CSL], BF16, tag=f"ss{g}", bufs=2,
                                     name="ss")
                        nc.gpsimd.tensor_mul(ss, g_rep[g], sinb_sb[:, sl])
                        ss_t[g] = ss
                    # DVE: cc + scans per tile; rotate-back deferred one
                    # tile so the Pool-made m2/m4 are ready when consumed.
                    cc_t = [None] * NT
                    newC = [None] * NT
                    newS = [None] * NT
                    m_t = [None] * NT   # (m1, m2, m3, m4) per tile
                    s_loc = [None] * NT

                    def rotate_back(g):
                        C, S = newC[g], newS[g]
                        m1 = a1.tile([128, CSL], BF16, tag="m1", bufs=2,
                                     name="m1")
                        nc.vector.tensor_mul(m1, C, cosb_sb[:, sl])
                        m2 = a1.tile([128, CSL], BF16, tag="m2", bufs=2,
                                     name="m2")
                        nc.gpsimd.tensor_mul(m2, S, sinb_sb[:, sl])
                        m3 = a1.tile([128, CSL], BF16, tag="m3", bufs=2,
                                     name="m3")
                        nc.vector.tensor_mul(m3, C, sinb_sb[:, sl])
                        m4 = a1.tile([128, CSL], BF16, tag="m4", bufs=2,
                                     name="m4")
                        if g % 2 == 0:
                            nc.gpsimd.tensor_mul(m4, S, cosb_sb[:, sl])
                        else:
                            nc.vector.tensor_mul(m4, S, cosb_sb[:, sl])
                        m_t[g] = (m1, m2, m3, m4)

                    def finish_tile(g):
                        m1, m2, m3, m4 = m_t[g]
                        s_r = a1.tile([128, CSL], BF16, tag=f"sr{g}", bufs=2,
                                      name=f"sr{g}")
                        nc.vector.tensor_add(s_r, m1, m2)
                        s_i = a1.tile([128, CSL], BF16, tag=f"si{g}", bufs=2,
                                      name=f"si{g}")
                        nc.vector.tensor_sub(s_i, m3, m4)
                        s_loc[g] = (s_r, s_i)

                    for g in range(NT):
                        cc = a1.tile([128, CSL], BF16, tag=f"cc{g}", bufs=2,
                                     name="cc")
                        nc.vector.tensor_mul(cc, g_rep[g], cosb_sb[:, sl])
                        cc_t[g] = cc
                        C = a1.tile([128, CSL], BF16, tag=f"C{g}", bufs=2,
                                    name="C")
                        nc.vector.tensor_tensor_scan(
                            C, _free_bcast(rho_sb[:, g:g + 1], CSL), cc,
                            initial=(init_sb[:, 2 * g:2 * g + 1] if cs == 0
                                     else C_t[g][:, CSL - 1:CSL]),
                            op0=AOT.mult, op1=AOT.add,
                        )
                        S = a1.tile([128, CSL], BF16, tag=f"S{g}", bufs=2,
                                    name="S")
                        nc.vector.tensor_tensor_scan(
                            S, _free_bcast(rho_sb[:, g:g + 1], CSL), ss_t[g],
                            initial=(init_sb[:, 2 * g + 1:2 * g + 2]
                                     if cs == 0
                                     else S_t[g][:, CSL - 1:CSL]),
                            op0=AOT.mult, op1=AOT.add,
                        )
                        newC[g], newS[g] = C, S
                        if g > 0:
                            rotate_back(g - 1)
                            finish_tile(g - 1)
                    rotate_back(NT - 1)
                    finish_tile(NT - 1)
                    C_t, S_t = newC, newS

                    # A2 for this compute slice: 4 psum tiles, waves of 2.
                    for w0 in range(0, CSL // 128, WAVE):
                        pss = [psum2.tile([128, OUT], FP32, tag="zm",
                                          bufs=2 * WAVE, name="ps_zm")
                               for _ in range(WAVE)]
                        for wi in range(WAVE):
                            nc.scalar.copy(pss[wi], bmixb_sb)
                        for g in range(NT):
                            for fld in range(2):
                                k = 2 * g + fld
                                for wi in range(WAVE):
                                    tch = w0 + wi
                                    nc.tensor.matmul(
                                        pss[wi],
                                        s_loc[g][fld][
                                            :, tch * 128:(tch + 1) * 128],
                                        wmix_sb[:, k, :],
                                        start=False,
                                        stop=(k == KCH - 1),
                                    )
                        for wi in range(WAVE):
                            zm_st = a1.tile([128, OUT], BF16, tag="zm_st",
                                            bufs=4, name="zm_st")
                            nc.scalar.copy(zm_st, pss[wi])
                            row0 = hst - SLS[grp][0] + (w0 + wi) * 128
                            nc.sync.dma_start(
                                zm_d[grp][row0:row0 + 128, :], zm_st,
                            )

                for grp, pos in RS_POS.items():
                    if pos >= NCS:
                        emit_rs(grp)

                # ---- phase B: LN + skip for own rows, 4 blocks ------------
                for bi, parts in enumerate(B_BLOCKS):
                    kt = None
                    zm_sb = pb.tile([128, OUT], BF16, tag="zm_sb",
                                    name="zm_sb")
                    p0 = 0
                    for (grp, off, n) in parts:
                        nc.sync.dma_start(
                            zm_sb[p0:p0 + n, :],
                            zm_own_d[grp][off:off + n, :],
                        )
                        p0 += n
                    # xb-row position of this block (host layout): block bi
                    # starts at row bi*128
                    kt = bi
                    v = pb.tile([128, OUT], BF16, tag="v", name="v")
                    nc.gpsimd.tensor_mul(v, zm_sb, gout_st[:, kt, :])
                    stats = pb.tile([128, 6], FP32, tag="stats", name="stats")
                    nc.vector.bn_stats(stats, v)
                    mv = pb.tile([128, 2], FP32, tag="mv", name="mv")
                    nc.vector.bn_aggr(mv, stats)
                    sd = pb.tile([128, 1], FP32, tag="sd", name="sd")
                    nc.scalar.activation(sd, mv[:, 1:2], AFT.Sqrt,
                                         bias=eps_sb)
                    rstd = pb.tile([128, 1], FP32, tag="rstd", name="rstd")
                    nc.vector.reciprocal(rstd, sd)
                    ln = pb.tile([128, OUT], BF16, tag="ln", name="ln")
                    nc.vector.tensor_scalar(
                        ln, v, mv[:, 0:1], rstd,
                        op0=AOT.subtract, op1=AOT.mult,
                    )
                    # t2 = (g_out - 1) * skip = -skip*(1-g_out)
                    t2 = pb.tile([128, OUT], BF16, tag="t2", name="t2")
                    nc.vector.scalar_tensor_tensor(
                        t2, gout_st[:, kt, :], 1.0, skip_st[:, kt, :],
                        op0=AOT.subtract, op1=AOT.mult,
                    )
                    res = pb.tile([128, OUT], FP32, tag="res", name="res")
                    nc.gpsimd.tensor_sub(res, ln, t2)
                    nc.sync.dma_start(outc[bi * 128:(bi + 1) * 128, :], res)

    nc.compile()
    return nc


def _xb_rows(c):
    """Global x-row indices, in xb/outc order, for core c (4 x 128)."""
    rows = []
    starts = [SLS[g][0] + c * BLS[g] for g in range(3)]
    for (grp, off, n) in [p for blk in B_BLOCKS for p in blk]:
        pass
    for blk in B_BLOCKS:
        for (grp, off, n) in blk:
            rows.extend(range(starts[grp] + off, starts[grp] + off + n))
    return np.array(rows)


def _prep_inputs(inputs):
    """Host-side: slice/rearrange FULL inputs into 8 per-core input maps."""
    x = np.asarray(inputs["x"], np.float32)
    state0 = np.asarray(inputs["state0"], np.float32)  # (1, TR, CTX, 2)
    a = np.abs(np.asarray(inputs["ffa_a"], np.float64))
    b = np.asarray(inputs["ffa_b"], np.float64)
    W_pre = np.asarray(inputs["W_pre"], np.float32)
    b_pre = np.asarray(inputs["b_pre"], np.float32)
    W_gin = np.asarray(inputs["W_gin"], np.float32)
    b_gin = np.asarray(inputs["b_gin"], np.float32)
    W_gout = np.asarray(inputs["W_gout"], np.float32)
    b_gout = np.asarray(inputs["b_gout"], np.float32)
    W_skip = np.asarray(inputs["W_skip"], np.float32)
    b_skip = np.asarray(inputs["b_skip"], np.float32)
    W_mix = np.asarray(inputs["W_mix"], np.float32)
    b_mix = np.asarray(inputs["b_mix"], np.float32)

    bf16 = mybir.dt.np(BF16)

    t_idx = np.arange(T, dtype=np.float64)
    ang = b[:, None] * t_idx[None, :]              # (CTX, T)
    cosb = np.tile(np.cos(ang), (2, 1)).astype(bf16)     # (128, T)
    sinb = np.tile(np.sin(ang), (2, 1)).astype(bf16)

    rho_v = np.exp(-a).astype(np.float32)          # (TR,)

    # scan initials from state0: R_{-1} = e^{i b_j} * s0 ; C init = Re,
    # S init = -Im (S-scan accumulates +sin terms, R_i = -S).
    s0r = state0[0, :, :, 0].astype(np.float64)    # (TR, CTX)
    s0i = state0[0, :, :, 1].astype(np.float64)
    cb1 = np.cos(b)[None, :]
    sb1 = np.sin(b)[None, :]
    initC = cb1 * s0r - sb1 * s0i                  # (TR, CTX)
    initS = -(sb1 * s0r + cb1 * s0i)

    # W_mix rows: row(i, j, re/im) = i*128 + fld*64 + j
    Wm = W_mix.reshape(TR, 2, CTX, OUT)            # [i][fld][j][o]

    xTb = np.ascontiguousarray(x.T.astype(bf16))   # (IN, T), same all cores
    wgout = W_gout.reshape(4, 128, OUT).astype(bf16)
    wskip = W_skip.reshape(4, 128, OUT).astype(bf16)
    ones_row = np.ones((1, 128), bf16)

    in_maps = []
    for c in range(NCORES):
        rho = np.empty((128, NT), np.float32)
        init_cs = np.empty((128, 2 * NT), np.float32)
        wmix = np.empty((KCH, 128, OUT), bf16)
        for g in range(NT):
            for il in range(2):
                tr = 8 * c + 2 * g + il
                sl = slice(il * 64, (il + 1) * 64)
                rho[sl, g] = rho_v[tr]
                init_cs[sl, 2 * g] = initC[tr]
                init_cs[sl, 2 * g + 1] = initS[tr]
                wmix[2 * g, sl] = Wm[tr, 0].astype(bf16)
                wmix[2 * g + 1, sl] = Wm[tr, 1].astype(bf16)
        trs = slice(8 * c, 8 * c + 8)
        Wpg = np.zeros((IN, 64), np.float32)
        Wpg[:, 0:TPC] = W_pre[:, trs]
        Wpg[:, 32:32 + TPC] = W_gin[:, trs]
        bias_pg_full = np.zeros((64, 1), np.float32)
        bias_pg_full[0:TPC, 0] = b_pre[trs]
        bias_pg_full[32:32 + TPC, 0] = b_gin[trs]
        xb = x[_xb_rows(c)]                              # (TL, IN)
        in_maps.append({
            "xT": xTb,
            "xbT": np.ascontiguousarray(xb.T.astype(bf16)),
            "wpg": Wpg.reshape(4, 128, 64).astype(bf16),
            "bias_pg": bias_pg_full,
            "cosb": cosb,
            "sinb": sinb,
            "rho": rho,
            "init_cs": init_cs,
            "wmix": wmix,
            "bmix": (b_mix if c == 0
                     else np.zeros_like(b_mix))[None, :].astype(np.float32),
            "wgout": wgout,
            "wskip": wskip,
            "bgout": b_gout[None, :].astype(bf16),
            "bskip": b_skip[None, :].astype(bf16),
            "ones_row": ones_row,
        })
    return in_maps


def _assemble(results) -> np.ndarray:
    """Scatter per-core outc rows back to their global x-row positions."""
    out = np.empty((T, OUT), np.float32)
    for c in range(NCORES):
        oc = np.asarray(results[c]["outc"])
        out[_xb_rows(c)] = oc
    return out


def _get_module(reps: int = 1):
    key = f"nc{reps}"
    if key not in _CACHE:
        _CACHE[key] = _build_module(reps)
    return _CACHE[key]


def kernel(**inputs) -> np.ndarray:
    nc = _get_module()
    in_maps = _prep_inputs(inputs)
    res = run_bass_kernel_spmd(nc, in_maps, list(range(NCORES)))
    return _assemble(res.results)


if __name__ == "__main__":
    import reference  # only available when run inside /root/problem
    inputs = reference.setup_inputs()
    out = kernel(**{k: np.asarray(v) for k, v in inputs.items()})
    print("kernel output", out.shape, out.dtype)


# revision 5
# speedup vs baseline: 1.1865x; 1.1865x over previous
"""FFM (fast-and-forgetful memory) layer on 8 Trainium2 NeuronCores.

Math: per (trace i, ctx j) channel, complex recurrence
    s_t = gamma_ij * s_{t-1} + z_t,   gamma_ij = exp(-|a_i|) * e^{i b_j}
with z_t = gated[t, i] broadcast over j, followed by
    zm = [Re s; Im s] @ W_mix + b_mix   (contraction over 2*64*64 = 8192)
    out = LN(zm * sigmoid(x@W_gout+b)) + (x@W_skip+b) * (1 - sigmoid(...))

Device decomposition (8 cores, trace-sharded; 3 ReduceScatters):
  A0 : every core computes gated ONLY for its own 8 traces over the FULL
       sequence (x^T streamed from DRAM).
  A1 : rotate the complex scan into two real scans R_t = rho*R_{t-1} + w_t
       (w = e^{-i b t} z) via DVE tensor_tensor_scan; rotation back with
       host-precomputed cos/sin(b_j t) bf16 tables. 8 compute slices of
       512 steps, scan state chained via direct initial=prev[:, -1:] APs.
       Engine split tuned to the cost model: scans + most muls on DVE,
       ss/m2(/m4 on even tiles) on gpsimd.
  A2 : zm partial (own 1024 real channels): per compute slice, 4 psum
       tiles of [128, OUT]; b_mix folded in via ACT psum pre-fill (no
       bias matmuls). bf16 stores; 3 grouped ReduceScatter(add) calls
       (cs 0-2 / 3-5 / 6-7) with issue positions tuned so the Pool queue
       never stalls on them.
  B  : gout/skip matmuls precomputed early; after each RS the core
       finishes LayerNorm + mix for its rows, in 4 row blocks laid out
       host-side so none spans a 128-partition boundary.
"""

import numpy as np
from contextlib import ExitStack

import concourse.bacc as bacc
import concourse.bass as bass
import concourse.tile as tile
from concourse import mybir
from concourse.bass_utils import run_bass_kernel_spmd

T, IN, TR, CTX, OUT = 4096, 512, 64, 64, 512
NCORES = 8
TL = T // NCORES        # 512: output rows per core
TPC = TR // NCORES      # 8 traces per core in the scan phase
NT = TPC // 2           # 4 channel tiles (2 traces x 64 ctx = 128 partitions)
KCH = 2 * NT            # 8 zm K-chunks per core (real+imag per tile)
CSL = 512               # compute-slice length (timesteps)
NCS = T // CSL          # 8 compute slices
GRP_CS = [(0, 3), (3, 6), (6, 8)]          # cs-ranges per RS group
SLS = [(lo * CSL, (hi - lo) * CSL) for lo, hi in GRP_CS]
BLS = [L // NCORES for _, L in SLS]        # (192, 192, 128) B rows / group
RS_POS = {0: 4, 1: 8, 2: 8}                # emit RS g before cs k (8 = end)
WAVE = 2                # psum groups per A2 wave
LN_EPS = 1e-6
FP32 = mybir.dt.float32
BF16 = mybir.dt.bfloat16
AOT = mybir.AluOpType
AFT = mybir.ActivationFunctionType

# B row blocks: (list of (group, row-offset-in-group, nrows)) per 128-block,
# laid out so each block sits in one 128-partition chunk of gout/skip/xb.
B_BLOCKS = [
    [(0, 0, 128)],
    [(1, 0, 128)],
    [(2, 0, 128)],
    [(0, 128, 64), (1, 128, 64)],
]

_CACHE: dict = {}


def _free_bcast(col: bass.AP, n: int) -> bass.AP:
    """Broadcast a [P, 1] column along the free dim to [P, n] via stride 0."""
    return bass.AP(tensor=col.tensor, offset=col.offset, ap=[col.ap[0], [0, n]])


def _build_module(reps: int = 1):
    nc = bacc.Bacc(
        "TRN2", target_bir_lowering=False, debug=False, num_devices=NCORES
    )

    def inp(name, shape, dt):
        return nc.dram_tensor(name, list(shape), dt, kind="ExternalInput").ap()

    xT = inp("xT", (IN, T), BF16)                  # full x, transposed
    xbT = inp("xbT", (IN, TL), BF16)               # x^T cols for B rows
    wpg = inp("wpg", (4, 128, 64), BF16)           # own pre @0..8, gin @32..40
    bias_pg = inp("bias_pg", (64, 1), FP32)        # own b_pre | b_gin
    cosb = inp("cosb", (128, T), BF16)             # cos(b_j t), 2x64 rows
    sinb = inp("sinb", (128, T), BF16)
    rho = inp("rho", (128, NT), FP32)              # exp(-|a_i|) per tile col
    init_cs = inp("init_cs", (128, 2 * NT), FP32)  # scan initials per tile
    wmix = inp("wmix", (KCH, 128, OUT), BF16)      # rearranged W_mix rows
    bmix = inp("bmix", (1, OUT), FP32)             # b_mix on core 0, else 0
    wgout = inp("wgout", (4, 128, OUT), BF16)
    wskip = inp("wskip", (4, 128, OUT), BF16)
    bgout = inp("bgout", (1, OUT), BF16)
    bskip = inp("bskip", (1, OUT), BF16)
    ones_row = inp("ones_row", (1, 128), BF16)

    outc = nc.dram_tensor("outc", [TL, OUT], FP32, kind="ExternalOutput").ap()

    groups = [list(range(NCORES))]

    with tile.TileContext(nc) as tc, ExitStack() as ctx:
        const = ctx.enter_context(tc.tile_pool(name="const", bufs=1))
        dram = ctx.enter_context(tc.tile_pool(name="dram", bufs=1, space="DRAM"))

        # ---- resident constants (heavy loads issued on the Pool queue:
        # DMA_SEQ_TIME is 25ns there vs 565+ elsewhere) ------------------
        rho_sb = const.tile([128, NT], FP32)
        nc.sync.dma_start(rho_sb, rho)
        init_sb = const.tile([128, 2 * NT], FP32)
        nc.sync.dma_start(init_sb, init_cs)
        bias_pg_sb = const.tile([64, 1], FP32)
        nc.sync.dma_start(bias_pg_sb, bias_pg)
        ones_sb = const.tile([1, 128], BF16)
        nc.sync.dma_start(ones_sb, ones_row)
        bgout_sb = const.tile([1, OUT], BF16)
        nc.sync.dma_start(bgout_sb, bgout)
        bskip_sb = const.tile([1, OUT], BF16)
        nc.sync.dma_start(bskip_sb, bskip)
        # b_mix broadcast to all 128 partitions (psum pre-fill source)
        bmixb_sb = const.tile([128, OUT], FP32)
        nc.sync.dma_start(
            bmixb_sb,
            bass.AP(tensor=bmix.tensor, offset=0, ap=[[0, 128], [1, OUT]]),
        )
        eps_sb = const.tile([128, 1], FP32)
        nc.vector.memset(eps_sb, LN_EPS)

        cosb_sb = const.tile([128, T], BF16)
        nc.sync.dma_start(cosb_sb, cosb)
        sinb_sb = const.tile([128, T], BF16)
        nc.sync.dma_start(sinb_sb, sinb)
        wpg_sb = const.tile([128, 4, 64], BF16)
        nc.scalar.dma_start(
            wpg_sb,
            bass.AP(tensor=wpg.tensor, offset=0,
                    ap=[[64, 128], [128 * 64, 4], [1, 64]]),
        )
        xb_sb = const.tile([128, 4, TL], BF16)
        nc.scalar.dma_start(
            xb_sb,
            bass.AP(tensor=xbT.tensor, offset=0,
                    ap=[[TL, 128], [128 * TL, 4], [1, TL]]),
        )
        wgout_sb = const.tile([128, 4, OUT], BF16)
        nc.scalar.dma_start(
            wgout_sb,
            bass.AP(tensor=wgout.tensor, offset=0,
                    ap=[[OUT, 128], [128 * OUT, 4], [1, OUT]]),
        )
        wskip_sb = const.tile([128, 4, OUT], BF16)
        nc.scalar.dma_start(
            wskip_sb,
            bass.AP(tensor=wskip.tensor, offset=0,
                    ap=[[OUT, 128], [128 * OUT, 4], [1, OUT]]),
        )
        wmix_sb = const.tile([128, KCH, OUT], BF16)
        nc.scalar.dma_start(
            wmix_sb,
            bass.AP(tensor=wmix.tensor, offset=0,
                    ap=[[OUT, 128], [128 * OUT, KCH], [1, OUT]]),
        )

        for _rep in range(reps):
            # ---- phase A0: gated for OWN 8 traces over full T -------------
            gbf = const.tile([TPC, T], BF16, tag="gbf")
            g_loc_d = dram.tile([TPC, T], BF16, name="g_loc_d")
            with tc.tile_pool(name="a0", bufs=4) as a0, \
                    tc.tile_pool(name="psa0", bufs=1, space="PSUM") as psum0:
                for tc8 in range(T // TL):
                    xt_t = a0.tile([128, 4, TL], BF16, tag="xt")
                    nc.sync.dma_start(
                        xt_t,
                        bass.AP(tensor=xT.tensor,
                                offset=tc8 * TL,
                                ap=[[T, 128], [128 * T, 4], [1, TL]]),
                    )
                    ps_pg = psum0.tile([64, TL], FP32, tag="pg", bufs=2)
                    for ki in range(4):
                        nc.tensor.matmul(
                            ps_pg,
                            wpg_sb[:, ki, :],
                            xt_t[:, ki, :],
                            start=(ki == 0),
                            stop=(ki == 3),
                        )
                    pre_sb = a0.tile([TPC, TL], FP32, tag="pre")
                    nc.scalar.activation(
                        pre_sb, ps_pg[0:TPC, :], AFT.Identity,
                        bias=bias_pg_sb[0:TPC, :],
                    )
                    sig_sb = a0.tile([TPC, TL], FP32, tag="sig")
                    nc.scalar.activation(
                        sig_sb, ps_pg[32:32 + TPC, :], AFT.Sigmoid,
                        bias=bias_pg_sb[32:32 + TPC, :],
                    )
                    nc.vector.tensor_mul(
                        gbf[:, tc8 * TL:(tc8 + 1) * TL], pre_sb, sig_sb
                    )
                    nc.sync.dma_start(
                        bass.AP(tensor=g_loc_d.tensor,
                                offset=g_loc_d.offset + tc8 * TL,
                                ap=[[T, TPC], [1, TL]]),
                        gbf[:, tc8 * TL:(tc8 + 1) * TL],
                    )

            # ---- early B-prep: gout/skip for this core's B rows -----------
            gout_st = const.tile([128, 4, OUT], BF16, tag="gout_st")
            skip_st = const.tile([128, 4, OUT], BF16, tag="skip_st")
            with tc.tile_pool(name="psb0", bufs=1, space="PSUM") as psb0:
                for kt in range(4):
                    tloc = kt * 128
                    ps_go = psb0.tile([128, OUT], FP32, tag="go", bufs=2,
                                      name="ps_go")
                    for ki in range(4):
                        nc.tensor.matmul(
                            ps_go,
                            xb_sb[:, ki, tloc:tloc + 128],
                            wgout_sb[:, ki, :],
                            start=(ki == 0),
                            stop=False,
                        )
                    nc.tensor.matmul(
                        ps_go, ones_sb, bgout_sb, start=False, stop=True,
                    )
                    nc.scalar.activation(gout_st[:, kt, :], ps_go,
                                         AFT.Sigmoid)
                    ps_sk = psb0.tile([128, OUT], FP32, tag="sk", bufs=2,
                                      name="ps_sk")
                    for ki in range(4):
                        nc.tensor.matmul(
                            ps_sk,
                            xb_sb[:, ki, tloc:tloc + 128],
                            wskip_sb[:, ki, :],
                            start=(ki == 0),
                            stop=False,
                        )
                    nc.tensor.matmul(
                        ps_sk, ones_sb, bskip_sb, start=False, stop=True,
                    )
                    nc.scalar.copy(skip_st[:, kt, :], ps_sk)

            # ---- phases A1 + A2 pipelined over 8 compute slices ----------
            with tc.tile_pool(name="a1", bufs=1) as a1, \
                    tc.tile_pool(name="psa2", bufs=1, space="PSUM") as psum2, \
                    tc.tile_pool(name="pb", bufs=2) as pb:
                zm_d = [dram.tile([SLS[g][1], OUT], BF16, name=f"zmd{g}")
                        for g in range(3)]
                zm_own_d = [dram.tile([BLS[g], OUT], BF16, name=f"zmo{g}")
                            for g in range(3)]

                C_t = [None] * NT     # per-tile C/S tiles for carry chaining
                S_t = [None] * NT

                def emit_rs(grp):
                    nc.gpsimd.collective_compute(
                        "ReduceScatter", AOT.add, replica_groups=groups,
                        ins=[zm_d[grp].opt()], outs=[zm_own_d[grp].opt()],
                    )

                for cs in range(NCS):
                    for grp, pos in RS_POS.items():
                        if pos == cs:
                            emit_rs(grp)
                    grp = next(gi for gi, (lo, hi) in enumerate(GRP_CS)
                               if lo <= cs < hi)
                    hst = cs * CSL
                    sl = slice(hst, hst + CSL)
                    g_rep = [None] * NT
                    # g_rep broadcast loads for all tiles first (Pool queue)
                    for g in range(NT):
                        gr = a1.tile([128, CSL], BF16, tag=f"grep{g}", bufs=2,
                                     name="g_rep")
                        for il in range(2):
                            nc.scalar.dma_start(
                                gr[il * CTX:(il + 1) * CTX, :],
                                bass.AP(
                                    tensor=g_loc_d.tensor,
                                    offset=(g_loc_d.offset
                                            + (2 * g + il) * T + hst),
                                    ap=[[0, CTX], [1, CSL]],
                                ),
                            )
                        g_rep[g] = gr
                    # Pool: ss for all tiles (feeds the S scans)
                    ss_t = [None] * NT
                    for g in range(NT):
                        ss = a1.tile([128, CSL], BF16, tag=f"ss{g}", bufs=2,
                                     name="ss")
                        nc.gpsimd.tensor_mul(ss, g_rep[g], sinb_sb[:, sl])
                        ss_t[g] = ss
                    # DVE: cc + scans per tile; rotate-back deferred one
                    # tile so the Pool-made m2/m4 are ready when consumed.
                    cc_t = [None] * NT
                    newC = [None] * NT
                    newS = [None] * NT
                    m_t = [None] * NT   # (m1, m2, m3, m4) per tile
                    s_loc = [None] * NT

                    def rotate_back(g):
                        C, S = newC[g], newS[g]
                        m1 = a1.tile([128, CSL], BF16, tag="m1", bufs=2,
                                     name="m1")
                        nc.vector.tensor_mul(m1, C, cosb_sb[:, sl])
                        m2 = a1.tile([128, CSL], BF16, tag="m2", bufs=2,
                                     name="m2")
                        nc.gpsimd.tensor_mul(m2, S, sinb_sb[:, sl])
                        m3 = a1.tile([128, CSL], BF16, tag="m3", bufs=2,
                                     name="m3")
                        nc.vector.tensor_mul(m3, C, sinb_sb[:, sl])
                        m4 = a1.tile([128, CSL], BF16, tag="m4", bufs=2,
                                     name="m4")
                        if g % 2 == 0:
                            nc.gpsimd.tensor_mul(m4, S, cosb_sb[:, sl])
                        else:
                            nc.vector.tensor_mul(m4, S, cosb_sb[:, sl])
                        m_t[g] = (m1, m2, m3, m4)

                    def finish_tile(g):
                        m1, m2, m3, m4 = m_t[g]
                        s_r = a1.tile([128, CSL], BF16, tag=f"sr{g}", bufs=2,
                                      name=f"sr{g}")
                        nc.vector.tensor_add(s_r, m1, m2)
                        s_i = a1.tile([128, CSL], BF16, tag=f"si{g}", bufs=2,
                                      name=f"si{g}")
                        nc.vector.tensor_sub(s_i, m3, m4)
                        s_loc[g] = (s_r, s_i)

                    for g in range(NT):
                        cc = a1.tile([128, CSL], BF16, tag=f"cc{g}", bufs=2,
                                     name="cc")
                        nc.vector.tensor_mul(cc, g_rep[g], cosb_sb[:, sl])
                        cc_t[g] = cc
                        C = a1.tile([128, CSL], BF16, tag=f"C{g}", bufs=2,
                                    name="C")
                        nc.vector.tensor_tensor_scan(
                            C, _free_bcast(rho_sb[:, g:g + 1], CSL), cc,
                            initial=(init_sb[:, 2 * g:2 * g + 1] if cs == 0
                                     else C_t[g][:, CSL - 1:CSL]),
                            op0=AOT.mult, op1=AOT.add,
                        )
                        S = a1.tile([128, CSL], BF16, tag=f"S{g}", bufs=2,
                                    name="S")
                        nc.vector.tensor_tensor_scan(
                            S, _free_bcast(rho_sb[:, g:g + 1], CSL), ss_t[g],
                            initial=(init_sb[:, 2 * g + 1:2 * g + 2]
                                     if cs == 0
                                     else S_t[g][:, CSL - 1:CSL]),
                            op0=AOT.mult, op1=AOT.add,
                        )
                        newC[g], newS[g] = C, S
                        if g > 0:
                            rotate_back(g - 1)
                            finish_tile(g - 1)
                    rotate_back(NT - 1)
                    finish_tile(NT - 1)
                    C_t, S_t = newC, newS

                    # A2 for this compute slice: 4 psum tiles, waves of 2.
                    for w0 in range(0, CSL // 128, WAVE):
                        pss = [psum2.tile([128, OUT], FP32, tag="zm",
                                          bufs=2 * WAVE, name="ps_zm")
                               for _ in range(WAVE)]
                        for wi in range(WAVE):
                            nc.scalar.copy(pss[wi], bmixb_sb)
                        for g in range(NT):
                            for fld in range(2):
                                k = 2 * g + fld
                                for wi in range(WAVE):
                                    tch = w0 + wi
                                    nc.tensor.matmul(
                                        pss[wi],
                                        s_loc[g][fld][
                                            :, tch * 128:(tch + 1) * 128],
                                        wmix_sb[:, k, :],
                                        start=False,
                                        stop=(k == KCH - 1),
                                    )
                        for wi in range(WAVE):
                            zm_st = a1.tile([128, OUT], BF16, tag="zm_st",
                                            bufs=4, name="zm_st")
                            nc.scalar.copy(zm_st, pss[wi])
                            row0 = hst - SLS[grp][0] + (w0 + wi) * 128
                            nc.sync.dma_start(
                                zm_d[grp][row0:row0 + 128, :], zm_st,
                            )

                for grp, pos in RS_POS.items():
                    if pos >= NCS:
                        emit_rs(grp)

                # ---- phase B: LN + skip for own rows, 4 blocks ------------
                for bi, parts in enumerate(B_BLOCKS):
                    kt = None
                    zm_sb = pb.tile([128, OUT], BF16, tag="zm_sb",
                                    name="zm_sb")
                    p0 = 0
                    for (grp, off, n) in parts:
                        nc.sync.dma_start(
                            zm_sb[p0:p0 + n, :],
                            zm_own_d[grp][off:off + n, :],
                        )
                        p0 += n
                    # xb-row position of this block (host layout): block bi
                    # starts at row bi*128
                    kt = bi
                    v = pb.tile([128, OUT], BF16, tag="v", name="v")
                    nc.gpsimd.tensor_mul(v, zm_sb, gout_st[:, kt, :])
                    stats = pb.tile([128, 6], FP32, tag="stats", name="stats")
                    nc.vector.bn_stats(stats, v)
                    mv = pb.tile([128, 2], FP32, tag="mv", name="mv")
                    nc.vector.bn_aggr(mv, stats)
                    sd = pb.tile([128, 1], FP32, tag="sd", name="sd")
                    nc.scalar.activation(sd, mv[:, 1:2], AFT.Sqrt,
                                         bias=eps_sb)
                    rstd = pb.tile([128, 1], FP32, tag="rstd", name="rstd")
                    nc.vector.reciprocal(rstd, sd)
                    ln = pb.tile([128, OUT], BF16, tag="ln", name="ln")
                    nc.vector.tensor_scalar(
                        ln, v, mv[:, 0:1], rstd,
                        op0=AOT.subtract, op1=AOT.mult,
                    )
                    # t2 = (g_out - 1) * skip = -skip*(1-g_out)
                    t2 = pb.tile([128, OUT], BF16, tag="t2", name="t2")
                    nc.vector.scalar_tensor_tensor(
                        t2, gout_st[:, kt, :], 1.0, skip_st[:, kt, :],
                        op0=AOT.subtract, op1=AOT.mult,
                    )
                    res = pb.tile([128, OUT], FP32, tag="res", name="res")
                    nc.gpsimd.tensor_sub(res, ln, t2)
                    nc.sync.dma_start(outc[bi * 128:(bi + 1) * 128, :], res)

    nc.compile()
    return nc


def _xb_rows(c):
    """Global x-row indices, in xb/outc order, for core c (4 x 128)."""
    rows = []
    starts = [SLS[g][0] + c * BLS[g] for g in range(3)]
    for (grp, off, n) in [p for blk in B_BLOCKS for p in blk]:
        pass
    for blk in B_BLOCKS:
        for (grp, off, n) in blk:
            rows.extend(range(starts[grp] + off, starts[grp] + off + n))
    return np.array(rows)


def _prep_inputs(inputs):
    """Host-side: slice/rearrange FULL inputs into 8 per-core input maps."""
    x = np.asarray(inputs["x"], np.float32)
    state0 = np.asarray(inputs["state0"], np.float32)  # (1, TR, CTX, 2)
    a = np.abs(np.asarray(inputs["ffa_a"], np.float64))
    b = np.asarray(inputs["ffa_b"], np.float64)
    W_pre = np.asarray(inputs["W_pre"], np.float32)
    b_pre = np.asarray(inputs["b_pre"], np.float32)
    W_gin = np.asarray(inputs["W_gin"], np.float32)
    b_gin = np.asarray(inputs["b_gin"], np.float32)
    W_gout = np.asarray(inputs["W_gout"], np.float32)
    b_gout = np.asarray(inputs["b_gout"], np.float32)
    W_skip = np.asarray(inputs["W_skip"], np.float32)
    b_skip = np.asarray(inputs["b_skip"], np.float32)
    W_mix = np.asarray(inputs["W_mix"], np.float32)
    b_mix = np.asarray(inputs["b_mix"], np.float32)

    bf16 = mybir.dt.np(BF16)

    t_idx = np.arange(T, dtype=np.float64)
    ang = b[:, None] * t_idx[None, :]              # (CTX, T)
    cosb = np.tile(np.cos(ang), (2, 1)).astype(bf16)     # (128, T)
    sinb = np.tile(np.sin(ang), (2, 1)).astype(bf16)

    rho_v = np.exp(-a).astype(np.float32)          # (TR,)

    # scan initials from state0: R_{-1} = e^{i b_j} * s0 ; C init = Re,
    # S init = -Im (S-scan accumulates +sin terms, R_i = -S).
    s0r = state0[0, :, :, 0].astype(np.float64)    # (TR, CTX)
    s0i = state0[0, :, :, 1].astype(np.float64)
    cb1 = np.cos(b)[None, :]
    sb1 = np.sin(b)[None, :]
    initC = cb1 * s0r - sb1 * s0i                  # (TR, CTX)
    initS = -(sb1 * s0r + cb1 * s0i)

    # W_mix rows: row(i, j, re/im) = i*128 + fld*64 + j
    Wm = W_mix.reshape(TR, 2, CTX, OUT)            # [i][fld][j][o]

    xTb = np.ascontiguousarray(x.T.astype(bf16))   # (IN, T), same all cores
    wgout = W_gout.reshape(4, 128, OUT).astype(bf16)
    wskip = W_skip.reshape(4, 128, OUT).astype(bf16)
    ones_row = np.ones((1, 128), bf16)

    in_maps = []
    for c in range(NCORES):
        rho = np.empty((128, NT), np.float32)
        init_cs = np.empty((128, 2 * NT), np.float32)
        wmix = np.empty((KCH, 128, OUT), bf16)
        for g in range(NT):
            for il in range(2):
                tr = 8 * c + 2 * g + il
                sl = slice(il * 64, (il + 1) * 64)
                rho[sl, g] = rho_v[tr]
                init_cs[sl, 2 * g] = initC[tr]
                init_cs[sl, 2 * g + 1] = initS[tr]
                wmix[2 * g, sl] = Wm[tr, 0].astype(bf16)
                wmix[2 * g + 1, sl] = Wm[tr, 1].astype(bf16)
        trs = slice(8 * c, 8 * c + 8)
        Wpg = np.zeros((IN, 64), np.float32)
        Wpg[:, 0:TPC] = W_pre[:, trs]
        Wpg[:, 32:32 + TPC] = W_gin[:, trs]
        bias_pg_full = np.zeros((64, 1), np.float32)
        bias_pg_full[0:TPC, 0] = b_pre[trs]
        bias_pg_full[32:32 + TPC, 0] = b_gin[trs]
        xb = x[_xb_rows(c)]                              # (TL, IN)
        in_maps.append({
            "xT": xTb,
            "xbT": np.ascontiguousarray(xb.T.astype(bf16)),
            "wpg": Wpg.reshape(4, 128, 64).astype(bf16),
            "bias_pg": bias_pg_full,
            "cosb": cosb,
            "sinb": sinb,
            "rho": rho,
            "init_cs": init_cs,
            "wmix": wmix,
            "bmix": (b_mix if c == 0
                     else np.zeros_like(b_mix))[None, :].astype(np.float32),
            "wgout": wgout,
            "wskip": wskip,
            "bgout": b_gout[None, :].astype(bf16),
            "bskip": b_skip[None, :].astype(bf16),
            "ones_row": ones_row,
        })
    return in_maps


def _assemble(results) -> np.ndarray:
    """Scatter per-core outc rows back to their global x-row positions."""
    out = np.empty((T, OUT), np.float32)
    for c in range(NCORES):
        oc = np.asarray(results[c]["outc"])
        out[_xb_rows(c)] = oc
    return out


def _get_module(reps: int = 1):
    key = f"nc{reps}"
    if key not in _CACHE:
        _CACHE[key] = _build_module(reps)
    return _CACHE[key]


def kernel(**inputs) -> np.ndarray:
    nc = _get_module()
    in_maps = _prep_inputs(inputs)
    res = run_bass_kernel_spmd(nc, in_maps, list(range(NCORES)))
    return _assemble(res.results)


if __name__ == "__main__":
    import reference  # only available when run inside /root/problem
    inputs = reference.setup_inputs()
    out = kernel(**{k: np.asarray(v) for k, v in inputs.items()})
    print("kernel output", out.shape, out.dtype)


# revision 7
# speedup vs baseline: 1.2039x; 1.0147x over previous
"""FFM (fast-and-forgetful memory) layer on 8 Trainium2 NeuronCores.

Math: per (trace i, ctx j) channel, complex recurrence
    s_t = gamma_ij * s_{t-1} + z_t,   gamma_ij = exp(-|a_i|) * e^{i b_j}
with z_t = gated[t, i] broadcast over j, followed by
    zm = [Re s; Im s] @ W_mix + b_mix   (contraction over 2*64*64 = 8192)
    out = LN(zm * sigmoid(x@W_gout+b)) + (x@W_skip+b) * (1 - sigmoid(...))

Device decomposition (8 cores, trace-sharded; 3 ReduceScatters):
  A0 : every core computes gated ONLY for its own 8 traces over the FULL
       sequence (x^T streamed from DRAM).
  A1 : rotate the complex scan into two real scans R_t = rho*R_{t-1} + w_t
       (w = e^{-i b t} z) via DVE tensor_tensor_scan; rotation back with
       host-precomputed cos/sin(b_j t) bf16 tables. 8 compute slices of
       512 steps, scan state chained via direct initial=prev[:, -1:] APs.
       Engine split tuned to the cost model: scans + most muls on DVE,
       ss/m2(/m4 on even tiles) on gpsimd.
  A2 : zm partial (own 1024 real channels): per compute slice, 4 psum
       tiles of [128, OUT]; b_mix folded in via ACT psum pre-fill (no
       bias matmuls). bf16 stores; 3 grouped ReduceScatter(add) calls
       (cs 0-2 / 3-5 / 6-7) with issue positions tuned so the Pool queue
       never stalls on them.
  B  : gout/skip matmuls precomputed early; after each RS the core
       finishes LayerNorm + mix for its rows, in 4 row blocks laid out
       host-side so none spans a 128-partition boundary.
"""

import numpy as np
from contextlib import ExitStack

import concourse.bacc as bacc
import concourse.bass as bass
import concourse.tile as tile
from concourse import mybir
from concourse.bass_utils import run_bass_kernel_spmd

T, IN, TR, CTX, OUT = 4096, 512, 64, 64, 512
NCORES = 8
TL = T // NCORES        # 512: output rows per core
TPC = TR // NCORES      # 8 traces per core in the scan phase
NT = TPC // 2           # 4 channel tiles (2 traces x 64 ctx = 128 partitions)
KCH = 2 * NT            # 8 zm K-chunks per core (real+imag per tile)
CSL = 512               # compute-slice length (timesteps)
NCS = T // CSL          # 8 compute slices
GRP_CS = [(0, 3), (3, 6), (6, 8)]          # cs-ranges per RS group
SLS = [(lo * CSL, (hi - lo) * CSL) for lo, hi in GRP_CS]
BLS = [L // NCORES for _, L in SLS]        # (192, 192, 128) B rows / group
RS_POS = {0: 4, 1: 8, 2: 8}                # emit RS g before cs k (8 = end)
WAVE = 2                # psum groups per A2 wave
LN_EPS = 1e-6
FP32 = mybir.dt.float32
BF16 = mybir.dt.bfloat16
AOT = mybir.AluOpType
AFT = mybir.ActivationFunctionType

# B row blocks: (list of (group, row-offset-in-group, nrows)) per 128-block,
# laid out so each block sits in one 128-partition chunk of gout/skip/xb.
B_BLOCKS = [
    [(0, 0, 128)],
    [(1, 0, 128)],
    [(2, 0, 128)],
    [(0, 128, 64), (1, 128, 64)],
]

_CACHE: dict = {}


def _free_bcast(col: bass.AP, n: int) -> bass.AP:
    """Broadcast a [P, 1] column along the free dim to [P, n] via stride 0."""
    return bass.AP(tensor=col.tensor, offset=col.offset, ap=[col.ap[0], [0, n]])


def _build_module(reps: int = 1):
    nc = bacc.Bacc(
        "TRN2", target_bir_lowering=False, debug=False, num_devices=NCORES
    )

    def inp(name, shape, dt):
        return nc.dram_tensor(name, list(shape), dt, kind="ExternalInput").ap()

    xT = inp("xT", (IN, T), BF16)                  # full x, transposed
    xbT = inp("xbT", (IN, TL), BF16)               # x^T cols for B rows
    wpg = inp("wpg", (4, 128, 64), BF16)           # own pre @0..8, gin @32..40
    bias_pg = inp("bias_pg", (64, 1), FP32)        # own b_pre | b_gin
    cosb = inp("cosb", (128, T), BF16)             # cos(b_j t), 2x64 rows
    sinb = inp("sinb", (128, T), BF16)
    rho = inp("rho", (128, NT), FP32)              # exp(-|a_i|) per tile col
    init_cs = inp("init_cs", (128, 2 * NT), FP32)  # scan initials per tile
    wmix = inp("wmix", (KCH, 128, OUT), BF16)      # rearranged W_mix rows
    bmix = inp("bmix", (1, OUT), FP32)             # b_mix on core 0, else 0
    wgout = inp("wgout", (4, 128, OUT), BF16)
    wskip = inp("wskip", (4, 128, OUT), BF16)
    bgout = inp("bgout", (1, OUT), BF16)
    bskip = inp("bskip", (1, OUT), BF16)
    ones_row = inp("ones_row", (1, 128), BF16)

    outc = nc.dram_tensor("outc", [TL, OUT], FP32, kind="ExternalOutput").ap()

    groups = [list(range(NCORES))]

    with tile.TileContext(nc) as tc, ExitStack() as ctx:
        const = ctx.enter_context(tc.tile_pool(name="const", bufs=1))
        dram = ctx.enter_context(tc.tile_pool(name="dram", bufs=1, space="DRAM"))

        # ---- resident constants (heavy loads issued on the Pool queue:
        # DMA_SEQ_TIME is 25ns there vs 565+ elsewhere) ------------------
        rho_sb = const.tile([128, NT], FP32)
        nc.sync.dma_start(rho_sb, rho)
        init_sb = const.tile([128, 2 * NT], FP32)
        nc.sync.dma_start(init_sb, init_cs)
        bias_pg_sb = const.tile([64, 1], FP32)
        nc.sync.dma_start(bias_pg_sb, bias_pg)
        ones_sb = const.tile([1, 128], BF16)
        nc.sync.dma_start(ones_sb, ones_row)
        bgout_sb = const.tile([1, OUT], BF16)
        nc.sync.dma_start(bgout_sb, bgout)
        bskip_sb = const.tile([1, OUT], BF16)
        nc.sync.dma_start(bskip_sb, bskip)
        # b_mix broadcast to all 128 partitions (psum pre-fill source)
        bmixb_sb = const.tile([128, OUT], FP32)
        nc.sync.dma_start(
            bmixb_sb,
            bass.AP(tensor=bmix.tensor, offset=0, ap=[[0, 128], [1, OUT]]),
        )
        eps_sb = const.tile([128, 1], FP32)
        nc.vector.memset(eps_sb, LN_EPS)

        cosb_sb = const.tile([128, T], BF16)
        nc.sync.dma_start(cosb_sb, cosb)
        sinb_sb = const.tile([128, T], BF16)
        nc.sync.dma_start(sinb_sb, sinb)
        wpg_sb = const.tile([128, 4, 64], BF16)
        nc.scalar.dma_start(
            wpg_sb,
            bass.AP(tensor=wpg.tensor, offset=0,
                    ap=[[64, 128], [128 * 64, 4], [1, 64]]),
        )
        xb_sb = const.tile([128, 4, TL], BF16)
        nc.scalar.dma_start(
            xb_sb,
            bass.AP(tensor=xbT.tensor, offset=0,
                    ap=[[TL, 128], [128 * TL, 4], [1, TL]]),
        )
        wgout_sb = const.tile([128, 4, OUT], BF16)
        nc.scalar.dma_start(
            wgout_sb,
            bass.AP(tensor=wgout.tensor, offset=0,
                    ap=[[OUT, 128], [128 * OUT, 4], [1, OUT]]),
        )
        wskip_sb = const.tile([128, 4, OUT], BF16)
        nc.scalar.dma_start(
            wskip_sb,
            bass.AP(tensor=wskip.tensor, offset=0,
                    ap=[[OUT, 128], [128 * OUT, 4], [1, OUT]]),
        )
        wmix_sb = const.tile([128, KCH, OUT], BF16)
        nc.scalar.dma_start(
            wmix_sb,
            bass.AP(tensor=wmix.tensor, offset=0,
                    ap=[[OUT, 128], [128 * OUT, KCH], [1, OUT]]),
        )

        for _rep in range(reps):
            # ---- phase A0: gated for OWN 8 traces over full T -------------
            gbf = const.tile([TPC, T], BF16, tag="gbf")
            g_loc_d = dram.tile([TPC, T], BF16, name="g_loc_d")
            with tc.tile_pool(name="a0", bufs=4) as a0, \
                    tc.tile_pool(name="psa0", bufs=1, space="PSUM") as psum0:
                xt_ts = []
                for tc8 in range(T // TL):
                    xt_t = a0.tile([128, 4, TL], BF16, tag=f"xt{tc8}",
                                   bufs=1)
                    nc.sync.dma_start(
                        xt_t,
                        bass.AP(tensor=xT.tensor,
                                offset=tc8 * TL,
                                ap=[[T, 128], [128 * T, 4], [1, TL]]),
                    )
                    xt_ts.append(xt_t)
                for tc8 in range(T // TL):
                    xt_t = xt_ts[tc8]
                    ps_pg = psum0.tile([64, TL], FP32, tag="pg", bufs=2)
                    for ki in range(4):
                        nc.tensor.matmul(
                            ps_pg,
                            wpg_sb[:, ki, :],
                            xt_t[:, ki, :],
                            start=(ki == 0),
                            stop=(ki == 3),
                        )
                    pre_sb = a0.tile([TPC, TL], FP32, tag="pre")
                    nc.scalar.activation(
                        pre_sb, ps_pg[0:TPC, :], AFT.Identity,
                        bias=bias_pg_sb[0:TPC, :],
                    )
                    sig_sb = a0.tile([TPC, TL], FP32, tag="sig")
                    nc.scalar.activation(
                        sig_sb, ps_pg[32:32 + TPC, :], AFT.Sigmoid,
                        bias=bias_pg_sb[32:32 + TPC, :],
                    )
                    nc.vector.tensor_mul(
                        gbf[:, tc8 * TL:(tc8 + 1) * TL], pre_sb, sig_sb
                    )
                    nc.scalar.dma_start(
                        bass.AP(tensor=g_loc_d.tensor,
                                offset=g_loc_d.offset + tc8 * TL,
                                ap=[[T, TPC], [1, TL]]),
                        gbf[:, tc8 * TL:(tc8 + 1) * TL],
                    )

            # ---- early B-prep: gout/skip for this core's B rows -----------
            gout_st = const.tile([128, 4, OUT], BF16, tag="gout_st")
            skip_st = const.tile([128, 4, OUT], BF16, tag="skip_st")
            with tc.tile_pool(name="psb0", bufs=1, space="PSUM") as psb0:
                for kt in range(4):
                    tloc = kt * 128
                    ps_go = psb0.tile([128, OUT], FP32, tag="go", bufs=2,
                                      name="ps_go")
                    for ki in range(4):
                        nc.tensor.matmul(
                            ps_go,
                            xb_sb[:, ki, tloc:tloc + 128],
                            wgout_sb[:, ki, :],
                            start=(ki == 0),
                            stop=False,
                        )
                    nc.tensor.matmul(
                        ps_go, ones_sb, bgout_sb, start=False, stop=True,
                    )
                    nc.scalar.activation(gout_st[:, kt, :], ps_go,
                                         AFT.Sigmoid)
                    ps_sk = psb0.tile([128, OUT], FP32, tag="sk", bufs=2,
                                      name="ps_sk")
                    for ki in range(4):
                        nc.tensor.matmul(
                            ps_sk,
                            xb_sb[:, ki, tloc:tloc + 128],
                            wskip_sb[:, ki, :],
                            start=(ki == 0),
                            stop=False,
                        )
                    nc.tensor.matmul(
                        ps_sk, ones_sb, bskip_sb, start=False, stop=True,
                    )
                    nc.scalar.copy(skip_st[:, kt, :], ps_sk)

            # ---- phases A1 + A2 pipelined over 8 compute slices ----------
            with tc.tile_pool(name="a1", bufs=1) as a1, \
                    tc.tile_pool(name="psa2", bufs=1, space="PSUM") as psum2, \
                    tc.tile_pool(name="pb", bufs=2) as pb:
                zm_d = [dram.tile([SLS[g][1], OUT], BF16, name=f"zmd{g}")
                        for g in range(3)]
                zm_own_d = [dram.tile([BLS[g], OUT], BF16, name=f"zmo{g}")
                            for g in range(3)]

                C_t = [None] * NT     # per-tile C/S tiles for carry chaining
                S_t = [None] * NT

                def emit_rs(grp):
                    nc.gpsimd.collective_compute(
                        "ReduceScatter", AOT.add, replica_groups=groups,
                        ins=[zm_d[grp].opt()], outs=[zm_own_d[grp].opt()],
                    )

                grep_tiles = {}

                def issue_grep(cs_i):
                    hst_i = cs_i * CSL
                    tiles = []
                    for g in range(NT):
                        gr = a1.tile([128, CSL], BF16, tag=f"grep{g}", bufs=2,
                                     name="g_rep")
                        for il in range(2):
                            nc.sync.dma_start(
                                gr[il * CTX:(il + 1) * CTX, :],
                                bass.AP(
                                    tensor=g_loc_d.tensor,
                                    offset=(g_loc_d.offset
                                            + (2 * g + il) * T + hst_i),
                                    ap=[[0, CTX], [1, CSL]],
                                ),
                            )
                        tiles.append(gr)
                    grep_tiles[cs_i] = tiles

                for cs in range(NCS):
                    for grp, pos in RS_POS.items():
                        if pos == cs:
                            emit_rs(grp)
                    grp = next(gi for gi, (lo, hi) in enumerate(GRP_CS)
                               if lo <= cs < hi)
                    hst = cs * CSL
                    sl = slice(hst, hst + CSL)
                    if cs == 0:
                        issue_grep(0)
                    if cs + 1 < NCS:
                        issue_grep(cs + 1)
                    g_rep = grep_tiles[cs]
                    # Pool: ss for all tiles (feeds the S scans)
                    ss_t = [None] * NT
                    for g in range(NT):
                        ss = a1.tile([128, CSL], BF16, tag=f"ss{g}", bufs=2,
                                     name="ss")
                        nc.gpsimd.tensor_mul(ss, g_rep[g], sinb_sb[:, sl])
                        ss_t[g] = ss
                    # DVE: cc + scans per tile; rotate-back deferred one
                    # tile so the Pool-made m2/m4 are ready when consumed.
                    cc_t = [None] * NT
                    newC = [None] * NT
                    newS = [None] * NT
                    m_t = [None] * NT   # (m1, m2, m3, m4) per tile
                    s_loc = [None] * NT

                    def rotate_back(g):
                        C, S = newC[g], newS[g]
                        m1 = a1.tile([128, CSL], BF16, tag="m1", bufs=2,
                                     name="m1")
                        nc.vector.tensor_mul(m1, C, cosb_sb[:, sl])
                        m2 = a1.tile([128, CSL], BF16, tag="m2", bufs=2,
                                     name="m2")
                        nc.gpsimd.tensor_mul(m2, S, sinb_sb[:, sl])
                        m3 = a1.tile([128, CSL], BF16, tag="m3", bufs=2,
                                     name="m3")
                        nc.vector.tensor_mul(m3, C, sinb_sb[:, sl])
                        m4 = a1.tile([128, CSL], BF16, tag="m4", bufs=2,
                                     name="m4")
                        if g % 2 == 0:
                            nc.gpsimd.tensor_mul(m4, S, cosb_sb[:, sl])
                        else:
                            nc.vector.tensor_mul(m4, S, cosb_sb[:, sl])
                        m_t[g] = (m1, m2, m3, m4)

                    def finish_tile(g):
                        m1, m2, m3, m4 = m_t[g]
                        s_r = a1.tile([128, CSL], BF16, tag=f"sr{g}", bufs=2,
                                      name=f"sr{g}")
                        nc.vector.tensor_add(s_r, m1, m2)
                        s_i = a1.tile([128, CSL], BF16, tag=f"si{g}", bufs=2,
                                      name=f"si{g}")
                        nc.vector.tensor_sub(s_i, m3, m4)
                        s_loc[g] = (s_r, s_i)

                    for g in range(NT):
                        cc = a1.tile([128, CSL], BF16, tag=f"cc{g}", bufs=2,
                                     name="cc")
                        nc.vector.tensor_mul(cc, g_rep[g], cosb_sb[:, sl])
                        cc_t[g] = cc
                        C = a1.tile([128, CSL], BF16, tag=f"C{g}", bufs=2,
                                    name="C")
                        nc.vector.tensor_tensor_scan(
                            C, _free_bcast(rho_sb[:, g:g + 1], CSL), cc,
                            initial=(init_sb[:, 2 * g:2 * g + 1] if cs == 0
                                     else C_t[g][:, CSL - 1:CSL]),
                            op0=AOT.mult, op1=AOT.add,
                        )
                        S = a1.tile([128, CSL], BF16, tag=f"S{g}", bufs=2,
                                    name="S")
                        nc.vector.tensor_tensor_scan(
                            S, _free_bcast(rho_sb[:, g:g + 1], CSL), ss_t[g],
                            initial=(init_sb[:, 2 * g + 1:2 * g + 2]
                                     if cs == 0
                                     else S_t[g][:, CSL - 1:CSL]),
                            op0=AOT.mult, op1=AOT.add,
                        )
                        newC[g], newS[g] = C, S
                        if g > 0:
                            rotate_back(g - 1)
                            finish_tile(g - 1)
                    rotate_back(NT - 1)
                    finish_tile(NT - 1)
                    C_t, S_t = newC, newS

                    # A2 for this compute slice: 4 psum tiles, waves of 2.
                    for w0 in range(0, CSL // 128, WAVE):
                        pss = [psum2.tile([128, OUT], FP32, tag="zm",
                                          bufs=2 * WAVE, name="ps_zm")
                               for _ in range(WAVE)]
                        for wi in range(WAVE):
                            nc.scalar.copy(pss[wi], bmixb_sb)
                        for g in range(NT):
                            for fld in range(2):
                                k = 2 * g + fld
                                for wi in range(WAVE):
                                    tch = w0 + wi
                                    nc.tensor.matmul(
                                        pss[wi],
                                        s_loc[g][fld][
                                            :, tch * 128:(tch + 1) * 128],
                                        wmix_sb[:, k, :],
                                        start=False,
                                        stop=(k == KCH - 1),
                                    )
                        for wi in range(WAVE):
                            zm_st = a1.tile([128, OUT], BF16, tag="zm_st",
                                            bufs=4, name="zm_st")
                            nc.scalar.copy(zm_st, pss[wi])
                            row0 = hst - SLS[grp][0] + (w0 + wi) * 128
                            nc.scalar.dma_start(
                                zm_d[grp][row0:row0 + 128, :], zm_st,
                            )

                for grp, pos in RS_POS.items():
                    if pos >= NCS:
                        emit_rs(grp)

                # ---- phase B: LN + skip for own rows, 4 blocks ------------
                for bi, parts in enumerate(B_BLOCKS):
                    kt = None
                    zm_sb = pb.tile([128, OUT], BF16, tag="zm_sb",
                                    name="zm_sb")
                    p0 = 0
                    for (grp, off, n) in parts:
                        nc.sync.dma_start(
                            zm_sb[p0:p0 + n, :],
                            zm_own_d[grp][off:off + n, :],
                        )
                        p0 += n
                    # xb-row position of this block (host layout): block bi
                    # starts at row bi*128
                    kt = bi
                    v = pb.tile([128, OUT], BF16, tag="v", name="v")
                    nc.gpsimd.tensor_mul(v, zm_sb, gout_st[:, kt, :])
                    stats = pb.tile([128, 6], FP32, tag="stats", name="stats")
                    nc.vector.bn_stats(stats, v)
                    mv = pb.tile([128, 2], FP32, tag="mv", name="mv")
                    nc.vector.bn_aggr(mv, stats)
                    sd = pb.tile([128, 1], FP32, tag="sd", name="sd")
                    nc.scalar.activation(sd, mv[:, 1:2], AFT.Sqrt,
                                         bias=eps_sb)
                    rstd = pb.tile([128, 1], FP32, tag="rstd", name="rstd")
                    nc.vector.reciprocal(rstd, sd)
                    ln = pb.tile([128, OUT], BF16, tag="ln", name="ln")
                    nc.vector.tensor_scalar(
                        ln, v, mv[:, 0:1], rstd,
                        op0=AOT.subtract, op1=AOT.mult,
                    )
                    # t2 = (g_out - 1) * skip = -skip*(1-g_out)
                    t2 = pb.tile([128, OUT], BF16, tag="t2", name="t2")
                    nc.vector.scalar_tensor_tensor(
                        t2, gout_st[:, kt, :], 1.0, skip_st[:, kt, :],
                        op0=AOT.subtract, op1=AOT.mult,
                    )
                    res = pb.tile([128, OUT], FP32, tag="res", name="res")
                    nc.gpsimd.tensor_sub(res, ln, t2)
                    nc.sync.dma_start(outc[bi * 128:(bi + 1) * 128, :], res)

    nc.compile()
    return nc


def _xb_rows(c):
    """Global x-row indices, in xb/outc order, for core c (4 x 128)."""
    rows = []
    starts = [SLS[g][0] + c * BLS[g] for g in range(3)]
    for (grp, off, n) in [p for blk in B_BLOCKS for p in blk]:
        pass
    for blk in B_BLOCKS:
        for (grp, off, n) in blk:
            rows.extend(range(starts[grp] + off, starts[grp] + off + n))
    return np.array(rows)


def _prep_inputs(inputs):
    """Host-side: slice/rearrange FULL inputs into 8 per-core input maps."""
    x = np.asarray(inputs["x"], np.float32)
    state0 = np.asarray(inputs["state0"], np.float32)  # (1, TR, CTX, 2)
    a = np.abs(np.asarray(inputs["ffa_a"], np.float64))
    b = np.asarray(inputs["ffa_b"], np.float64)
    W_pre = np.asarray(inputs["W_pre"], np.float32)
    b_pre = np.asarray(inputs["b_pre"], np.float32)
    W_gin = np.asarray(inputs["W_gin"], np.float32)
    b_gin = np.asarray(inputs["b_gin"], np.float32)
    W_gout = np.asarray(inputs["W_gout"], np.float32)
    b_gout = np.asarray(inputs["b_gout"], np.float32)
    W_skip = np.asarray(inputs["W_skip"], np.float32)
    b_skip = np.asarray(inputs["b_skip"], np.float32)
    W_mix = np.asarray(inputs["W_mix"], np.float32)
    b_mix = np.asarray(inputs["b_mix"], np.float32)

    bf16 = mybir.dt.np(BF16)

    t_idx = np.arange(T, dtype=np.float64)
    ang = b[:, None] * t_idx[None, :]              # (CTX, T)
    cosb = np.tile(np.cos(ang), (2, 1)).astype(bf16)     # (128, T)
    sinb = np.tile(np.sin(ang), (2, 1)).astype(bf16)

    rho_v = np.exp(-a).astype(np.float32)          # (TR,)

    # scan initials from state0: R_{-1} = e^{i b_j} * s0 ; C init = Re,
    # S init = -Im (S-scan accumulates +sin terms, R_i = -S).
    s0r = state0[0, :, :, 0].astype(np.float64)    # (TR, CTX)
    s0i = state0[0, :, :, 1].astype(np.float64)
    cb1 = np.cos(b)[None, :]
    sb1 = np.sin(b)[None, :]
    initC = cb1 * s0r - sb1 * s0i                  # (TR, CTX)
    initS = -(sb1 * s0r + cb1 * s0i)

    # W_mix rows: row(i, j, re/im) = i*128 + fld*64 + j
    Wm = W_mix.reshape(TR, 2, CTX, OUT)            # [i][fld][j][o]

    xTb = np.ascontiguousarray(x.T.astype(bf16))   # (IN, T), same all cores
    wgout = W_gout.reshape(4, 128, OUT).astype(bf16)
    wskip = W_skip.reshape(4, 128, OUT).astype(bf16)
    ones_row = np.ones((1, 128), bf16)

    in_maps = []
    for c in range(NCORES):
        rho = np.empty((128, NT), np.float32)
        init_cs = np.empty((128, 2 * NT), np.float32)
        wmix = np.empty((KCH, 128, OUT), bf16)
        for g in range(NT):
            for il in range(2):
                tr = 8 * c + 2 * g + il
                sl = slice(il * 64, (il + 1) * 64)
                rho[sl, g] = rho_v[tr]
                init_cs[sl, 2 * g] = initC[tr]
                init_cs[sl, 2 * g + 1] = initS[tr]
                wmix[2 * g, sl] = Wm[tr, 0].astype(bf16)
                wmix[2 * g + 1, sl] = Wm[tr, 1].astype(bf16)
        trs = slice(8 * c, 8 * c + 8)
        Wpg = np.zeros((IN, 64), np.float32)
        Wpg[:, 0:TPC] = W_pre[:, trs]
        Wpg[:, 32:32 + TPC] = W_gin[:, trs]
        bias_pg_full = np.zeros((64, 1), np.float32)
        bias_pg_full[0:TPC, 0] = b_pre[trs]
        bias_pg_full[32:32 + TPC, 0] = b_gin[trs]
        xb = x[_xb_rows(c)]                              # (TL, IN)
        in_maps.append({
            "xT": xTb,
            "xbT": np.ascontiguousarray(xb.T.astype(bf16)),
            "wpg": Wpg.reshape(4, 128, 64).astype(bf16),
            "bias_pg": bias_pg_full,
            "cosb": cosb,
            "sinb": sinb,
            "rho": rho,
            "init_cs": init_cs,
            "wmix": wmix,
            "bmix": (b_mix if c == 0
                     else np.zeros_like(b_mix))[None, :].astype(np.float32),
            "wgout": wgout,
            "wskip": wskip,
            "bgout": b_gout[None, :].astype(bf16),
            "bskip": b_skip[None, :].astype(bf16),
            "ones_row": ones_row,
        })
    return in_maps


def _assemble(results) -> np.ndarray:
    """Scatter per-core outc rows back to their global x-row positions."""
    out = np.empty((T, OUT), np.float32)
    for c in range(NCORES):
        oc = np.asarray(results[c]["outc"])
        out[_xb_rows(c)] = oc
    return out


def _get_module(reps: int = 1):
    key = f"nc{reps}"
    if key not in _CACHE:
        _CACHE[key] = _build_module(reps)
    return _CACHE[key]


def kernel(**inputs) -> np.ndarray:
    nc = _get_module()
    in_maps = _prep_inputs(inputs)
    res = run_bass_kernel_spmd(nc, in_maps, list(range(NCORES)))
    return _assemble(res.results)


if __name__ == "__main__":
    import reference  # only available when run inside /root/problem
    inputs = reference.setup_inputs()
    out = kernel(**{k: np.asarray(v) for k, v in inputs.items()})
    print("kernel output", out.shape, out.dtype)


# revision 9
# speedup vs baseline: 1.3218x; 1.0979x over previous
"""FFM (fast-and-forgetful memory) layer on 8 Trainium2 NeuronCores.

Math: per (trace i, ctx j) channel, complex recurrence
    s_t = gamma_ij * s_{t-1} + z_t,   gamma_ij = exp(-|a_i|) * e^{i b_j}
with z_t = gated[t, i] broadcast over j, followed by
    zm = [Re s; Im s] @ W_mix + b_mix   (contraction over 2*64*64 = 8192)
    out = LN(zm * sigmoid(x@W_gout+b)) + (x@W_skip+b) * (1 - sigmoid(...))

Device decomposition (8 cores, trace-sharded; 3 ReduceScatters):
  A0 : every core computes gated ONLY for its own 8 traces over the FULL
       sequence (x^T streamed from DRAM).
  A1 : rotate the complex scan into two real scans R_t = rho*R_{t-1} + w_t
       (w = e^{-i b t} z) via DVE tensor_tensor_scan; rotation back with
       host-precomputed cos/sin(b_j t) bf16 tables. 8 compute slices of
       512 steps, scan state chained via direct initial=prev[:, -1:] APs.
       Engine split tuned to the cost model: scans + most muls on DVE,
       ss/m2(/m4 on even tiles) on gpsimd.
  A2 : zm partial (own 1024 real channels): per compute slice, 4 psum
       tiles of [128, OUT]; b_mix folded in via ACT psum pre-fill (no
       bias matmuls). bf16 stores; 3 grouped ReduceScatter(add) calls
       (cs 0-2 / 3-5 / 6-7) with issue positions tuned so the Pool queue
       never stalls on them.
  B  : gout/skip matmuls precomputed early; after each RS the core
       finishes LayerNorm + mix for its rows, in 4 row blocks laid out
       host-side so none spans a 128-partition boundary.
"""

import numpy as np
from contextlib import ExitStack

import concourse.bacc as bacc
import concourse.bass as bass
import concourse.tile as tile
from concourse import mybir
from concourse.bass_utils import run_bass_kernel_spmd

T, IN, TR, CTX, OUT = 4096, 512, 64, 64, 512
NCORES = 8
TL = T // NCORES        # 512: output rows per core
TPC = TR // NCORES      # 8 traces per core in the scan phase
NT = TPC // 2           # 4 channel tiles (2 traces x 64 ctx = 128 partitions)
KCH = 2 * NT            # 8 zm K-chunks per core (real+imag per tile)
CSL = 512               # compute-slice length (timesteps)
NCS = T // CSL          # 8 compute slices
GRP_CS = [(0, 3), (3, 6), (6, 8)]          # cs-ranges per RS group
SLS = [(lo * CSL, (hi - lo) * CSL) for lo, hi in GRP_CS]
BLS = [L // NCORES for _, L in SLS]        # (192, 192, 128) B rows / group
RS_POS = {0: 4, 1: 8, 2: 8}                # emit RS g after cs k's ew (8=end)
BPREP_POS = 3           # emit gout/skip prep after this cs
B_ORDER = [0, 1, 3, 2]  # B blocks ordered by RS availability
WAVE = 2                # psum groups per A2 wave
LN_EPS = 1e-6
FP32 = mybir.dt.float32
BF16 = mybir.dt.bfloat16
AOT = mybir.AluOpType
AFT = mybir.ActivationFunctionType

# B row blocks: (list of (group, row-offset-in-group, nrows)) per 128-block,
# laid out so each block sits in one 128-partition chunk of gout/skip/xb.
B_BLOCKS = [
    [(0, 0, 128)],
    [(1, 0, 128)],
    [(2, 0, 128)],
    [(0, 128, 64), (1, 128, 64)],
]

_CACHE: dict = {}


def _free_bcast(col: bass.AP, n: int) -> bass.AP:
    """Broadcast a [P, 1] column along the free dim to [P, n] via stride 0."""
    return bass.AP(tensor=col.tensor, offset=col.offset, ap=[col.ap[0], [0, n]])


def _build_module(reps: int = 1):
    nc = bacc.Bacc(
        "TRN2", target_bir_lowering=False, debug=False, num_devices=NCORES
    )

    def inp(name, shape, dt):
        return nc.dram_tensor(name, list(shape), dt, kind="ExternalInput").ap()

    xT = inp("xT", (IN, T), BF16)                  # full x, transposed
    xbT = inp("xbT", (IN, TL), BF16)               # x^T cols for B rows
    wpg = inp("wpg", (4, 128, 64), BF16)           # own pre @0..8, gin @32..40
    bias_pg = inp("bias_pg", (64, 1), FP32)        # own b_pre | b_gin
    cossin = inp("cossin", (128, 2 * T), BF16)     # cos(b_j t) | sin(b_j t)
    rho = inp("rho", (128, NT), FP32)              # exp(-|a_i|) per tile col
    init_cs = inp("init_cs", (128, 2 * NT), FP32)  # scan initials per tile
    prepl = inp("prepl", (NT, 8, 128), BF16)       # replication patterns
    wmix = inp("wmix", (KCH, 128, OUT), BF16)      # rearranged W_mix rows
    bmixb = inp("bmixb", (128, OUT), FP32)         # b_mix bcast (core0) | 0
    wgout = inp("wgout", (4, 128, OUT), BF16)
    wskip = inp("wskip", (4, 128, OUT), BF16)
    bgout = inp("bgout", (1, OUT), BF16)
    bskip = inp("bskip", (1, OUT), BF16)
    ones_row = inp("ones_row", (1, 128), BF16)

    outc = nc.dram_tensor("outc", [TL, OUT], FP32, kind="ExternalOutput").ap()

    groups = [list(range(NCORES))]

    with tile.TileContext(nc) as tc, ExitStack() as ctx:
        const = ctx.enter_context(tc.tile_pool(name="const", bufs=1))
        dram = ctx.enter_context(tc.tile_pool(name="dram", bufs=1, space="DRAM"))

        # ---- resident constants, ordered for earliest need ---------------
        wpg_sb = const.tile([128, 4, 64], BF16)
        nc.sync.dma_start(
            wpg_sb,
            bass.AP(tensor=wpg.tensor, offset=0,
                    ap=[[64, 128], [128 * 64, 4], [1, 64]]),
        )
        rho_sb = const.tile([128, NT], FP32)
        nc.sync.dma_start(rho_sb, rho)
        init_sb = const.tile([128, 2 * NT], FP32)
        nc.sync.dma_start(init_sb, init_cs)
        bias_pg_sb = const.tile([64, 1], FP32)
        nc.sync.dma_start(bias_pg_sb, bias_pg)
        prepl_sb = const.tile([8, NT, 128], BF16)
        nc.sync.dma_start(
            prepl_sb,
            bass.AP(tensor=prepl.tensor, offset=0,
                    ap=[[128, 8], [8 * 128, NT], [1, 128]]),
        )
        eps_sb = const.tile([128, 1], FP32)
        nc.vector.memset(eps_sb, LN_EPS)

        xt_ts = []
        with tc.tile_pool(name="a0buf", bufs=1) as a0buf:
            for tc8 in range(2):
                xt_t = a0buf.tile([128, 4, TL], BF16, tag=f"xt{tc8}")
                nc.sync.dma_start(
                    xt_t,
                    bass.AP(tensor=xT.tensor, offset=tc8 * TL,
                            ap=[[T, 128], [128 * T, 4], [1, TL]]),
                )
                xt_ts.append(xt_t)
            cs_sb = const.tile([128, 2 * T], BF16)
            cosb_sb = cs_sb[:, 0:T]
            sinb_sb = cs_sb[:, T:2 * T]
            nc.sync.dma_start(cs_sb, cossin)
            for tc8 in range(2, T // TL):
                xt_t = a0buf.tile([128, 4, TL], BF16, tag=f"xt{tc8}")
                nc.sync.dma_start(
                    xt_t,
                    bass.AP(tensor=xT.tensor, offset=tc8 * TL,
                            ap=[[T, 128], [128 * T, 4], [1, TL]]),
                )
                xt_ts.append(xt_t)
            bmixb_sb = const.tile([128, OUT], FP32)
            nc.sync.dma_start(bmixb_sb, bmixb)
            wmix_sb = const.tile([128, KCH, OUT], BF16)
            nc.sync.dma_start(
                wmix_sb,
                bass.AP(tensor=wmix.tensor, offset=0,
                        ap=[[OUT, 128], [128 * OUT, KCH], [1, OUT]]),
            )
            xb_sb = const.tile([128, 4, TL], BF16)
            nc.scalar.dma_start(
                xb_sb,
                bass.AP(tensor=xbT.tensor, offset=0,
                        ap=[[TL, 128], [128 * TL, 4], [1, TL]]),
            )
            wgout_sb = const.tile([128, 4, OUT], BF16)
            nc.scalar.dma_start(
                wgout_sb,
                bass.AP(tensor=wgout.tensor, offset=0,
                        ap=[[OUT, 128], [128 * OUT, 4], [1, OUT]]),
            )
            wskip_sb = const.tile([128, 4, OUT], BF16)
            nc.scalar.dma_start(
                wskip_sb,
                bass.AP(tensor=wskip.tensor, offset=0,
                        ap=[[OUT, 128], [128 * OUT, 4], [1, OUT]]),
            )
            ones_sb = const.tile([1, 128], BF16)
            nc.scalar.dma_start(ones_sb, ones_row)
            bgout_sb = const.tile([1, OUT], BF16)
            nc.scalar.dma_start(bgout_sb, bgout)
            bskip_sb = const.tile([1, OUT], BF16)
            nc.scalar.dma_start(bskip_sb, bskip)

            for _rep in range(reps):
                gbf = const.tile([TPC, T], BF16, tag="gbf")
                gout_st = const.tile([128, 4, OUT], BF16, tag="gout_st")
                skip_st = const.tile([128, 4, OUT], BF16, tag="skip_st")
                with tc.tile_pool(name="a1", bufs=1) as a1, \
                        tc.tile_pool(name="ps", bufs=1, space="PSUM") as psum, \
                        tc.tile_pool(name="pb", bufs=2) as pb:
                    zm_d = [dram.tile([SLS[g][1], OUT], BF16, name=f"zmd{g}")
                            for g in range(3)]
                    zm_own_d = [dram.tile([BLS[g], OUT], BF16,
                                          name=f"zmo{g}")
                                for g in range(3)]

                    def emit_a0(k):
                        """gated for timesteps [k*TL, (k+1)*TL) -> gbf."""
                        ps_pg = psum.tile([64, TL], FP32, tag="pg", bufs=2)
                        for ki in range(4):
                            nc.tensor.matmul(
                                ps_pg, wpg_sb[:, ki, :], xt_ts[k][:, ki, :],
                                start=(ki == 0), stop=(ki == 3),
                            )
                        pre_sb = a1.tile([TPC, TL], FP32, tag="pre", bufs=2,
                                         name="pre")
                        nc.scalar.activation(
                            pre_sb, ps_pg[0:TPC, :], AFT.Identity,
                            bias=bias_pg_sb[0:TPC, :],
                        )
                        sig_sb = a1.tile([TPC, TL], FP32, tag="sig", bufs=2,
                                         name="sig")
                        nc.scalar.activation(
                            sig_sb, ps_pg[32:32 + TPC, :], AFT.Sigmoid,
                            bias=bias_pg_sb[32:32 + TPC, :],
                        )
                        nc.vector.tensor_mul(
                            gbf[:, k * TL:(k + 1) * TL], pre_sb, sig_sb
                        )

                    def emit_repl(k):
                        """g_rep for slice k via PE replication -> SBUF."""
                        tiles = []
                        for g in range(NT):
                            ps_g = psum.tile([128, CSL], FP32, tag="repl",
                                             bufs=2, name="ps_g")
                            nc.tensor.matmul(
                                ps_g, prepl_sb[:, g, :],
                                gbf[:, k * CSL:(k + 1) * CSL],
                                start=True, stop=True,
                            )
                            g_sb = a1.tile([128, CSL], BF16, tag=f"g{g}",
                                           bufs=2, name="g_sb")
                            nc.scalar.copy(g_sb, ps_g)
                            tiles.append(g_sb)
                        grep_tiles[k] = tiles

                    def emit_bprep():
                        for kt in range(4):
                            tloc = kt * 128
                            ps_go = psum.tile([128, OUT], FP32, tag="zm",
                                              bufs=4, name="ps_go")
                            for ki in range(4):
                                nc.tensor.matmul(
                                    ps_go, xb_sb[:, ki, tloc:tloc + 128],
                                    wgout_sb[:, ki, :],
                                    start=(ki == 0), stop=False,
                                )
                            nc.tensor.matmul(
                                ps_go, ones_sb, bgout_sb,
                                start=False, stop=True,
                            )
                            nc.scalar.activation(gout_st[:, kt, :], ps_go,
                                                 AFT.Sigmoid)
                            ps_sk = psum.tile([128, OUT], FP32, tag="zm",
                                              bufs=4, name="ps_sk")
                            for ki in range(4):
                                nc.tensor.matmul(
                                    ps_sk, xb_sb[:, ki, tloc:tloc + 128],
                                    wskip_sb[:, ki, :],
                                    start=(ki == 0), stop=False,
                                )
                            nc.tensor.matmul(
                                ps_sk, ones_sb, bskip_sb,
                                start=False, stop=True,
                            )
                            nc.scalar.copy(skip_st[:, kt, :], ps_sk)

                    def emit_rs(grp):
                        nc.gpsimd.collective_compute(
                            "ReduceScatter", AOT.add, replica_groups=groups,
                            ins=[zm_d[grp].opt()], outs=[zm_own_d[grp].opt()],
                        )

                    grep_tiles = {}
                    C_t = [None] * NT
                    S_t = [None] * NT

                    def emit_ew(cs):
                        """elementwise + scans for slice cs (DVE/Pool)."""
                        nonlocal C_t, S_t
                        sl = slice(cs * CSL, (cs + 1) * CSL)
                        g_rep = grep_tiles[cs]
                        ss_t = [None] * NT
                        for g in range(NT):
                            ss = a1.tile([128, CSL], BF16, tag=f"ss{g}",
                                         bufs=2, name="ss")
                            nc.gpsimd.tensor_mul(ss, g_rep[g], sinb_sb[:, sl])
                            ss_t[g] = ss
                        newC = [None] * NT
                        newS = [None] * NT
                        m_t = [None] * NT
                        s_loc = [None] * NT

                        def rotate_back(g):
                            C, S = newC[g], newS[g]
                            m1 = a1.tile([128, CSL], BF16, tag="m1", bufs=2,
                                         name="m1")
                            nc.vector.tensor_mul(m1, C, cosb_sb[:, sl])
                            m2 = a1.tile([128, CSL], BF16, tag="m2", bufs=2,
                                         name="m2")
                            nc.gpsimd.tensor_mul(m2, S, sinb_sb[:, sl])
                            m3 = a1.tile([128, CSL], BF16, tag="m3", bufs=2,
                                         name="m3")
                            nc.vector.tensor_mul(m3, C, sinb_sb[:, sl])
                            m4 = a1.tile([128, CSL], BF16, tag="m4", bufs=2,
                                         name="m4")
                            if g % 2 == 0:
                                nc.gpsimd.tensor_mul(m4, S, cosb_sb[:, sl])
                            else:
                                nc.vector.tensor_mul(m4, S, cosb_sb[:, sl])
                            m_t[g] = (m1, m2, m3, m4)

                        def finish_tile(g):
                            m1, m2, m3, m4 = m_t[g]
                            s_r = a1.tile([128, CSL], BF16, tag=f"sr{g}",
                                          bufs=2, name=f"sr{g}")
                            nc.vector.tensor_add(s_r, m1, m2)
                            s_i = a1.tile([128, CSL], BF16, tag=f"si{g}",
                                          bufs=2, name=f"si{g}")
                            nc.vector.tensor_sub(s_i, m3, m4)
                            s_loc[g] = (s_r, s_i)

                        for g in range(NT):
                            cc = a1.tile([128, CSL], BF16, tag=f"cc{g}",
                                         bufs=2, name="cc")
                            nc.vector.tensor_mul(cc, g_rep[g],
                                                 cosb_sb[:, sl])
                            C = a1.tile([128, CSL], BF16, tag=f"C{g}",
                                        bufs=2, name="C")
                            nc.vector.tensor_tensor_scan(
                                C, _free_bcast(rho_sb[:, g:g + 1], CSL), cc,
                                initial=(init_sb[:, 2 * g:2 * g + 1]
                                         if cs == 0
                                         else C_t[g][:, CSL - 1:CSL]),
                                op0=AOT.mult, op1=AOT.add,
                            )
                            S = a1.tile([128, CSL], BF16, tag=f"S{g}",
                                        bufs=2, name="S")
                            nc.vector.tensor_tensor_scan(
                                S, _free_bcast(rho_sb[:, g:g + 1], CSL),
                                ss_t[g],
                                initial=(init_sb[:, 2 * g + 1:2 * g + 2]
                                         if cs == 0
                                         else S_t[g][:, CSL - 1:CSL]),
                                op0=AOT.mult, op1=AOT.add,
                            )
                            newC[g], newS[g] = C, S
                            if g > 0:
                                rotate_back(g - 1)
                                finish_tile(g - 1)
                        rotate_back(NT - 1)
                        finish_tile(NT - 1)
                        C_t, S_t = newC, newS
                        return s_loc

                    def emit_a2(cs, s_loc):
                        grp = next(gi for gi, (lo, hi) in enumerate(GRP_CS)
                                   if lo <= cs < hi)
                        zm_st = a1.tile([128, 4, OUT], BF16, tag="zm_st",
                                        bufs=2, name="zm_st")
                        for w0 in range(0, CSL // 128, WAVE):
                            pss = [psum.tile([128, OUT], FP32, tag="zm",
                                             bufs=4, name="ps_zm")
                                   for _ in range(WAVE)]
                            for wi in range(WAVE):
                                nc.scalar.copy(pss[wi], bmixb_sb)
                            for g in range(NT):
                                for fld in range(2):
                                    k = 2 * g + fld
                                    for wi in range(WAVE):
                                        tch = w0 + wi
                                        nc.tensor.matmul(
                                            pss[wi],
                                            s_loc[g][fld][
                                                :, tch * 128:(tch + 1) * 128],
                                            wmix_sb[:, k, :],
                                            start=False,
                                            stop=(k == KCH - 1),
                                        )
                            for wi in range(WAVE):
                                nc.scalar.copy(zm_st[:, w0 + wi, :], pss[wi])
                        row0 = cs * CSL - SLS[grp][0]
                        nc.scalar.dma_start(
                            bass.AP(tensor=zm_d[grp].tensor,
                                    offset=zm_d[grp].offset + row0 * OUT,
                                    ap=[[OUT, 128], [128 * OUT, 4],
                                        [1, OUT]]),
                            zm_st,
                        )

                    # ---- fused pipeline ---------------------------------
                    emit_a0(0)
                    emit_repl(0)
                    for cs in range(NCS):
                        if cs + 1 < NCS:
                            emit_a0(cs + 1)
                        s_loc = emit_ew(cs)
                        if cs + 1 < NCS:
                            emit_repl(cs + 1)
                        if cs == BPREP_POS:
                            emit_bprep()
                        for grp, pos in RS_POS.items():
                            if pos == cs:
                                emit_rs(grp)
                        emit_a2(cs, s_loc)
                    for grp, pos in RS_POS.items():
                        if pos >= NCS:
                            emit_rs(grp)

                    # ---- phase B: LN + skip, 4 row blocks ---------------
                    res_t = pb.tile([128, 4, OUT], FP32, tag="res",
                                    name="res")
                    for bi in B_ORDER:
                        parts = B_BLOCKS[bi]
                        zm_sb = pb.tile([128, OUT], BF16, tag="zm_sb",
                                        name="zm_sb")
                        p0 = 0
                        for (grp, off, n) in parts:
                            nc.sync.dma_start(
                                zm_sb[p0:p0 + n, :],
                                zm_own_d[grp][off:off + n, :],
                            )
                            p0 += n
                        kt = bi
                        v = pb.tile([128, OUT], BF16, tag="v", name="v")
                        nc.gpsimd.tensor_mul(v, zm_sb, gout_st[:, kt, :])
                        stats = pb.tile([128, 6], FP32, tag="stats",
                                        name="stats")
                        nc.vector.bn_stats(stats, v)
                        mv = pb.tile([128, 2], FP32, tag="mv", name="mv")
                        nc.vector.bn_aggr(mv, stats)
                        sd = pb.tile([128, 1], FP32, tag="sd", name="sd")
                        nc.scalar.activation(sd, mv[:, 1:2], AFT.Sqrt,
                                             bias=eps_sb)
                        rstd = pb.tile([128, 1], FP32, tag="rstd",
                                       name="rstd")
                        nc.vector.reciprocal(rstd, sd)
                        ln = pb.tile([128, OUT], BF16, tag="ln", name="ln")
                        nc.vector.tensor_scalar(
                            ln, v, mv[:, 0:1], rstd,
                            op0=AOT.subtract, op1=AOT.mult,
                        )
                        t2 = pb.tile([128, OUT], BF16, tag="t2", name="t2")
                        nc.vector.scalar_tensor_tensor(
                            t2, gout_st[:, kt, :], 1.0, skip_st[:, kt, :],
                            op0=AOT.subtract, op1=AOT.mult,
                        )
                        nc.gpsimd.tensor_sub(res_t[:, bi, :], ln, t2)
                        nc.sync.dma_start(
                            outc[bi * 128:(bi + 1) * 128, :],
                            res_t[:, bi, :],
                        )

    nc.compile()
    return nc


def _xb_rows(c):
    """Global x-row indices, in xb/outc order, for core c (4 x 128)."""
    rows = []
    starts = [SLS[g][0] + c * BLS[g] for g in range(3)]
    for (grp, off, n) in [p for blk in B_BLOCKS for p in blk]:
        pass
    for blk in B_BLOCKS:
        for (grp, off, n) in blk:
            rows.extend(range(starts[grp] + off, starts[grp] + off + n))
    return np.array(rows)


def _prep_inputs(inputs):
    """Host-side: slice/rearrange FULL inputs into 8 per-core input maps."""
    x = np.asarray(inputs["x"], np.float32)
    state0 = np.asarray(inputs["state0"], np.float32)  # (1, TR, CTX, 2)
    a = np.abs(np.asarray(inputs["ffa_a"], np.float64))
    b = np.asarray(inputs["ffa_b"], np.float64)
    W_pre = np.asarray(inputs["W_pre"], np.float32)
    b_pre = np.asarray(inputs["b_pre"], np.float32)
    W_gin = np.asarray(inputs["W_gin"], np.float32)
    b_gin = np.asarray(inputs["b_gin"], np.float32)
    W_gout = np.asarray(inputs["W_gout"], np.float32)
    b_gout = np.asarray(inputs["b_gout"], np.float32)
    W_skip = np.asarray(inputs["W_skip"], np.float32)
    b_skip = np.asarray(inputs["b_skip"], np.float32)
    W_mix = np.asarray(inputs["W_mix"], np.float32)
    b_mix = np.asarray(inputs["b_mix"], np.float32)

    bf16 = mybir.dt.np(BF16)

    t_idx = np.arange(T, dtype=np.float64)
    ang = b[:, None] * t_idx[None, :]              # (CTX, T)
    cosb = np.tile(np.cos(ang), (2, 1)).astype(bf16)     # (128, T)
    sinb = np.tile(np.sin(ang), (2, 1)).astype(bf16)
    cossin = np.concatenate([cosb, sinb], axis=1)        # (128, 2T)
    prepl = np.zeros((NT, 8, 128), bf16)           # replication patterns
    for g in range(NT):
        prepl[g, 2 * g, 0:64] = 1.0
        prepl[g, 2 * g + 1, 64:128] = 1.0

    rho_v = np.exp(-a).astype(np.float32)          # (TR,)

    # scan initials from state0: R_{-1} = e^{i b_j} * s0 ; C init = Re,
    # S init = -Im (S-scan accumulates +sin terms, R_i = -S).
    s0r = state0[0, :, :, 0].astype(np.float64)    # (TR, CTX)
    s0i = state0[0, :, :, 1].astype(np.float64)
    cb1 = np.cos(b)[None, :]
    sb1 = np.sin(b)[None, :]
    initC = cb1 * s0r - sb1 * s0i                  # (TR, CTX)
    initS = -(sb1 * s0r + cb1 * s0i)

    # W_mix rows: row(i, j, re/im) = i*128 + fld*64 + j
    Wm = W_mix.reshape(TR, 2, CTX, OUT)            # [i][fld][j][o]

    xTb = np.ascontiguousarray(x.T.astype(bf16))   # (IN, T), same all cores
    wgout = W_gout.reshape(4, 128, OUT).astype(bf16)
    wskip = W_skip.reshape(4, 128, OUT).astype(bf16)
    ones_row = np.ones((1, 128), bf16)

    in_maps = []
    for c in range(NCORES):
        rho = np.empty((128, NT), np.float32)
        init_cs = np.empty((128, 2 * NT), np.float32)
        wmix = np.empty((KCH, 128, OUT), bf16)
        for g in range(NT):
            for il in range(2):
                tr = 8 * c + 2 * g + il
                sl = slice(il * 64, (il + 1) * 64)
                rho[sl, g] = rho_v[tr]
                init_cs[sl, 2 * g] = initC[tr]
                init_cs[sl, 2 * g + 1] = initS[tr]
                wmix[2 * g, sl] = Wm[tr, 0].astype(bf16)
                wmix[2 * g + 1, sl] = Wm[tr, 1].astype(bf16)
        trs = slice(8 * c, 8 * c + 8)
        Wpg = np.zeros((IN, 64), np.float32)
        Wpg[:, 0:TPC] = W_pre[:, trs]
        Wpg[:, 32:32 + TPC] = W_gin[:, trs]
        bias_pg_full = np.zeros((64, 1), np.float32)
        bias_pg_full[0:TPC, 0] = b_pre[trs]
        bias_pg_full[32:32 + TPC, 0] = b_gin[trs]
        xb = x[_xb_rows(c)]                              # (TL, IN)
        bmixb = np.broadcast_to(
            (b_mix if c == 0 else np.zeros_like(b_mix))[None, :],
            (128, OUT)).astype(np.float32).copy()
        in_maps.append({
            "xT": xTb,
            "xbT": np.ascontiguousarray(xb.T.astype(bf16)),
            "wpg": Wpg.reshape(4, 128, 64).astype(bf16),
            "bias_pg": bias_pg_full,
            "cossin": cossin,
            "rho": rho,
            "init_cs": init_cs,
            "prepl": prepl,
            "wmix": wmix,
            "bmixb": bmixb,
            "wgout": wgout,
            "wskip": wskip,
            "bgout": b_gout[None, :].astype(bf16),
            "bskip": b_skip[None, :].astype(bf16),
            "ones_row": ones_row,
        })
    return in_maps


def _assemble(results) -> np.ndarray:
    """Scatter per-core outc rows back to their global x-row positions."""
    out = np.empty((T, OUT), np.float32)
    for c in range(NCORES):
        oc = np.asarray(results[c]["outc"])
        out[_xb_rows(c)] = oc
    return out


def _get_module(reps: int = 1):
    key = f"nc{reps}"
    if key not in _CACHE:
        _CACHE[key] = _build_module(reps)
    return _CACHE[key]


def kernel(**inputs) -> np.ndarray:
    nc = _get_module()
    in_maps = _prep_inputs(inputs)
    res = run_bass_kernel_spmd(nc, in_maps, list(range(NCORES)))
    return _assemble(res.results)


if __name__ == "__main__":
    import reference  # only available when run inside /root/problem
    inputs = reference.setup_inputs()
    out = kernel(**{k: np.asarray(v) for k, v in inputs.items()})
    print("kernel output", out.shape, out.dtype)


# revision 10
# speedup vs baseline: 1.3778x; 1.0423x over previous
"""FFM (fast-and-forgetful memory) layer on 8 Trainium2 NeuronCores.

Math: per (trace i, ctx j) channel, complex recurrence
    s_t = gamma_ij * s_{t-1} + z_t,   gamma_ij = exp(-|a_i|) * e^{i b_j}
with z_t = gated[t, i] broadcast over j, followed by
    zm = [Re s; Im s] @ W_mix + b_mix   (contraction over 2*64*64 = 8192)
    out = LN(zm * sigmoid(x@W_gout+b)) + (x@W_skip+b) * (1 - sigmoid(...))

Device decomposition (8 cores, trace-sharded; 3 ReduceScatters):
  A0 : every core computes gated ONLY for its own 8 traces over the FULL
       sequence (x^T streamed from DRAM).
  A1 : rotate the complex scan into two real scans R_t = rho*R_{t-1} + w_t
       (w = e^{-i b t} z) via DVE tensor_tensor_scan; rotation back with
       host-precomputed cos/sin(b_j t) bf16 tables. 8 compute slices of
       512 steps, scan state chained via direct initial=prev[:, -1:] APs.
       Engine split tuned to the cost model: scans + most muls on DVE,
       ss/m2(/m4 on even tiles) on gpsimd.
  A2 : zm partial (own 1024 real channels): per compute slice, 4 psum
       tiles of [128, OUT]; b_mix folded in via ACT psum pre-fill (no
       bias matmuls). bf16 stores; 3 grouped ReduceScatter(add) calls
       (cs 0-2 / 3-5 / 6-7) with issue positions tuned so the Pool queue
       never stalls on them.
  B  : gout/skip matmuls precomputed early; after each RS the core
       finishes LayerNorm + mix for its rows, in 4 row blocks laid out
       host-side so none spans a 128-partition boundary.
"""

import numpy as np
from contextlib import ExitStack

import concourse.bacc as bacc
import concourse.bass as bass
import concourse.tile as tile
from concourse import mybir
from concourse.bass_utils import run_bass_kernel_spmd

T, IN, TR, CTX, OUT = 4096, 512, 64, 64, 512
NCORES = 8
TL = T // NCORES        # 512: output rows per core
TPC = TR // NCORES      # 8 traces per core in the scan phase
NT = TPC // 2           # 4 channel tiles (2 traces x 64 ctx = 128 partitions)
KCH = 2 * NT            # 8 zm K-chunks per core (real+imag per tile)
CSL = 512               # compute-slice length (timesteps)
NCS = T // CSL          # 8 compute slices
GRP_CS = [(0, 3), (3, 6), (6, 8)]          # cs-ranges per RS group
SLS = [(lo * CSL, (hi - lo) * CSL) for lo, hi in GRP_CS]
BLS = [L // NCORES for _, L in SLS]        # (192, 192, 128) B rows / group
RS_POS = {0: 3, 1: 6, 2: 8}                # emit RS g after cs k's ew (8=end)
BPREP_POS = 3           # emit gout/skip prep after this cs
B_ORDER = [0, 1, 3, 2]  # B blocks ordered by RS availability
WAVE = 2                # psum groups per A2 wave
LN_EPS = 1e-6
FP32 = mybir.dt.float32
BF16 = mybir.dt.bfloat16
AOT = mybir.AluOpType
AFT = mybir.ActivationFunctionType

# B row blocks: (list of (group, row-offset-in-group, nrows)) per 128-block,
# laid out so each block sits in one 128-partition chunk of gout/skip/xb.
B_BLOCKS = [
    [(0, 0, 128)],
    [(1, 0, 128)],
    [(2, 0, 128)],
    [(0, 128, 64), (1, 128, 64)],
]

_CACHE: dict = {}


def _free_bcast(col: bass.AP, n: int) -> bass.AP:
    """Broadcast a [P, 1] column along the free dim to [P, n] via stride 0."""
    return bass.AP(tensor=col.tensor, offset=col.offset, ap=[col.ap[0], [0, n]])


def _build_module(reps: int = 1):
    nc = bacc.Bacc(
        "TRN2", target_bir_lowering=False, debug=False, num_devices=NCORES
    )

    def inp(name, shape, dt):
        return nc.dram_tensor(name, list(shape), dt, kind="ExternalInput").ap()

    xT = inp("xT", (IN, T), BF16)                  # full x, transposed
    xbT = inp("xbT", (IN, TL), BF16)               # x^T cols for B rows
    wpg = inp("wpg", (4, 128, 64), BF16)           # own pre @0..8, gin @32..40
    bias_pg = inp("bias_pg", (64, 1), FP32)        # own b_pre | b_gin
    cossin = inp("cossin", (128, 2 * T), BF16)     # cos(b_j t) | sin(b_j t)
    rho = inp("rho", (128, NT), FP32)              # exp(-|a_i|) per tile col
    init_cs = inp("init_cs", (128, 2 * NT), FP32)  # scan initials per tile
    prepl = inp("prepl", (NT, 8, 128), BF16)       # replication patterns
    wmix = inp("wmix", (KCH, 128, OUT), BF16)      # rearranged W_mix rows
    bmixb = inp("bmixb", (128, OUT), FP32)         # b_mix bcast (core0) | 0
    wgout = inp("wgout", (4, 128, OUT), BF16)
    wskip = inp("wskip", (4, 128, OUT), BF16)
    bgout = inp("bgout", (1, OUT), BF16)
    bskip = inp("bskip", (1, OUT), BF16)
    ones_row = inp("ones_row", (1, 128), BF16)

    outc = nc.dram_tensor("outc", [TL, OUT], FP32, kind="ExternalOutput").ap()

    groups = [list(range(NCORES))]

    with tile.TileContext(nc) as tc, ExitStack() as ctx:
        const = ctx.enter_context(tc.tile_pool(name="const", bufs=1))
        dram = ctx.enter_context(tc.tile_pool(name="dram", bufs=1, space="DRAM"))

        # ---- resident constants, ordered for earliest need (all on SP so
        # the shared HWDGE serves them in emission order) -----------------
        wpg_sb = const.tile([128, 4, 64], BF16)
        nc.sync.dma_start(
            wpg_sb,
            bass.AP(tensor=wpg.tensor, offset=0,
                    ap=[[64, 128], [128 * 64, 4], [1, 64]]),
        )
        cs_sb = const.tile([128, 2 * T], BF16)
        cosb_sb = cs_sb[:, 0:T]
        sinb_sb = cs_sb[:, T:2 * T]
        nc.sync.dma_start(cosb_sb, bass.AP(tensor=cossin.tensor, offset=0,
                                           ap=[[2 * T, 128], [1, T]]))
        eps_sb = const.tile([128, 1], FP32)
        nc.vector.memset(eps_sb, LN_EPS)
        # warm the ACT function tables before the pipeline needs them
        warm_sb = const.tile([1, 1], FP32)
        nc.scalar.activation(warm_sb, eps_sb[0:1, :], AFT.Sigmoid)
        nc.scalar.activation(warm_sb, eps_sb[0:1, :], AFT.Sqrt)

        xt_ts = []
        with tc.tile_pool(name="a0buf", bufs=1) as a0buf:
            def load_xt(tc8):
                xt_t = a0buf.tile([128, 4, TL], BF16, tag=f"xt{tc8}")
                nc.sync.dma_start(
                    xt_t,
                    bass.AP(tensor=xT.tensor, offset=tc8 * TL,
                            ap=[[T, 128], [128 * T, 4], [1, TL]]),
                )
                xt_ts.append(xt_t)

            load_xt(0)
            nc.sync.dma_start(sinb_sb,
                              bass.AP(tensor=cossin.tensor, offset=T,
                                      ap=[[2 * T, 128], [1, T]]))
            load_xt(1)
            rho_sb = const.tile([128, NT], FP32)
            nc.sync.dma_start(rho_sb, rho)
            init_sb = const.tile([128, 2 * NT], FP32)
            nc.sync.dma_start(init_sb, init_cs)
            bias_pg_sb = const.tile([64, 1], FP32)
            nc.sync.dma_start(bias_pg_sb, bias_pg)
            prepl_sb = const.tile([8, NT, 128], BF16)
            nc.sync.dma_start(
                prepl_sb,
                bass.AP(tensor=prepl.tensor, offset=0,
                        ap=[[128, 8], [8 * 128, NT], [1, 128]]),
            )
            bmixb_sb = const.tile([128, OUT], FP32)
            nc.sync.dma_start(bmixb_sb, bmixb)
            load_xt(2)
            load_xt(3)
            wmix_sb = const.tile([128, KCH, OUT], BF16)
            nc.sync.dma_start(
                wmix_sb,
                bass.AP(tensor=wmix.tensor, offset=0,
                        ap=[[OUT, 128], [128 * OUT, KCH], [1, OUT]]),
            )
            for tc8 in range(4, T // TL):
                load_xt(tc8)
            xb_sb = const.tile([128, 4, TL], BF16)
            nc.sync.dma_start(
                xb_sb,
                bass.AP(tensor=xbT.tensor, offset=0,
                        ap=[[TL, 128], [128 * TL, 4], [1, TL]]),
            )
            wgout_sb = const.tile([128, 4, OUT], BF16)
            nc.sync.dma_start(
                wgout_sb,
                bass.AP(tensor=wgout.tensor, offset=0,
                        ap=[[OUT, 128], [128 * OUT, 4], [1, OUT]]),
            )
            wskip_sb = const.tile([128, 4, OUT], BF16)
            nc.sync.dma_start(
                wskip_sb,
                bass.AP(tensor=wskip.tensor, offset=0,
                        ap=[[OUT, 128], [128 * OUT, 4], [1, OUT]]),
            )
            ones_sb = const.tile([1, 128], BF16)
            nc.sync.dma_start(ones_sb, ones_row)
            bgout_sb = const.tile([1, OUT], BF16)
            nc.sync.dma_start(bgout_sb, bgout)
            bskip_sb = const.tile([1, OUT], BF16)
            nc.sync.dma_start(bskip_sb, bskip)

            for _rep in range(reps):
                gbf = const.tile([TPC, T], BF16, tag="gbf")
                gout_st = const.tile([128, 4, OUT], BF16, tag="gout_st")
                skip_st = const.tile([128, 4, OUT], BF16, tag="skip_st")
                with tc.tile_pool(name="a1", bufs=1) as a1, \
                        tc.tile_pool(name="ps", bufs=1, space="PSUM") as psum, \
                        tc.tile_pool(name="pb", bufs=2) as pb:
                    zm_d = [dram.tile([SLS[g][1], OUT], BF16, name=f"zmd{g}")
                            for g in range(3)]
                    zm_own_d = [dram.tile([BLS[g], OUT], BF16,
                                          name=f"zmo{g}")
                                for g in range(3)]

                    def emit_a0(k):
                        """gated for timesteps [k*TL, (k+1)*TL) -> gbf."""
                        ps_pg = psum.tile([64, TL], FP32, tag="pg", bufs=2)
                        for ki in range(4):
                            nc.tensor.matmul(
                                ps_pg, wpg_sb[:, ki, :], xt_ts[k][:, ki, :],
                                start=(ki == 0), stop=(ki == 3),
                            )
                        pre_sb = a1.tile([TPC, TL], FP32, tag="pre", bufs=2,
                                         name="pre")
                        nc.scalar.activation(
                            pre_sb, ps_pg[0:TPC, :], AFT.Identity,
                            bias=bias_pg_sb[0:TPC, :],
                        )
                        sig_sb = a1.tile([TPC, TL], FP32, tag="sig", bufs=2,
                                         name="sig")
                        nc.scalar.activation(
                            sig_sb, ps_pg[32:32 + TPC, :], AFT.Sigmoid,
                            bias=bias_pg_sb[32:32 + TPC, :],
                        )
                        nc.vector.tensor_mul(
                            gbf[:, k * TL:(k + 1) * TL], pre_sb, sig_sb
                        )

                    def emit_repl(k):
                        """g_rep for slice k via PE replication -> SBUF."""
                        tiles = []
                        for g in range(NT):
                            ps_g = psum.tile([128, CSL], FP32, tag="repl",
                                             bufs=2, name="ps_g")
                            nc.tensor.matmul(
                                ps_g, prepl_sb[:, g, :],
                                gbf[:, k * CSL:(k + 1) * CSL],
                                start=True, stop=True,
                            )
                            g_sb = a1.tile([128, CSL], BF16, tag=f"g{g}",
                                           bufs=2, name="g_sb")
                            nc.scalar.copy(g_sb, ps_g)
                            tiles.append(g_sb)
                        grep_tiles[k] = tiles

                    def emit_bprep():
                        for kt in range(4):
                            tloc = kt * 128
                            ps_go = psum.tile([128, OUT], FP32, tag="zm",
                                              bufs=4, name="ps_go")
                            for ki in range(4):
                                nc.tensor.matmul(
                                    ps_go, xb_sb[:, ki, tloc:tloc + 128],
                                    wgout_sb[:, ki, :],
                                    start=(ki == 0), stop=False,
                                )
                            nc.tensor.matmul(
                                ps_go, ones_sb, bgout_sb,
                                start=False, stop=True,
                            )
                            nc.scalar.activation(gout_st[:, kt, :], ps_go,
                                                 AFT.Sigmoid)
                            ps_sk = psum.tile([128, OUT], FP32, tag="zm",
                                              bufs=4, name="ps_sk")
                            for ki in range(4):
                                nc.tensor.matmul(
                                    ps_sk, xb_sb[:, ki, tloc:tloc + 128],
                                    wskip_sb[:, ki, :],
                                    start=(ki == 0), stop=False,
                                )
                            nc.tensor.matmul(
                                ps_sk, ones_sb, bskip_sb,
                                start=False, stop=True,
                            )
                            nc.scalar.copy(skip_st[:, kt, :], ps_sk)

                    def emit_rs(grp):
                        nc.gpsimd.collective_compute(
                            "ReduceScatter", AOT.add, replica_groups=groups,
                            ins=[zm_d[grp].opt()], outs=[zm_own_d[grp].opt()],
                        )

                    grep_tiles = {}
                    C_t = [None] * NT
                    S_t = [None] * NT

                    def emit_ew(cs):
                        """elementwise + scans for slice cs (DVE/Pool)."""
                        nonlocal C_t, S_t
                        sl = slice(cs * CSL, (cs + 1) * CSL)
                        g_rep = grep_tiles[cs]
                        ss_t = [None] * NT
                        for g in range(NT):
                            ss = a1.tile([128, CSL], BF16, tag=f"ss{g}",
                                         bufs=2, name="ss")
                            nc.gpsimd.tensor_mul(ss, g_rep[g], sinb_sb[:, sl])
                            ss_t[g] = ss
                        newC = [None] * NT
                        newS = [None] * NT
                        m_t = [None] * NT
                        s_loc = [None] * NT

                        def rotate_back(g):
                            C, S = newC[g], newS[g]
                            m1 = a1.tile([128, CSL], BF16, tag="m1", bufs=2,
                                         name="m1")
                            nc.vector.tensor_mul(m1, C, cosb_sb[:, sl])
                            m2 = a1.tile([128, CSL], BF16, tag="m2", bufs=2,
                                         name="m2")
                            nc.gpsimd.tensor_mul(m2, S, sinb_sb[:, sl])
                            m3 = a1.tile([128, CSL], BF16, tag="m3", bufs=2,
                                         name="m3")
                            nc.vector.tensor_mul(m3, C, sinb_sb[:, sl])
                            m4 = a1.tile([128, CSL], BF16, tag="m4", bufs=2,
                                         name="m4")
                            if g % 2 == 0:
                                nc.gpsimd.tensor_mul(m4, S, cosb_sb[:, sl])
                            else:
                                nc.vector.tensor_mul(m4, S, cosb_sb[:, sl])
                            m_t[g] = (m1, m2, m3, m4)

                        def finish_tile(g):
                            m1, m2, m3, m4 = m_t[g]
                            s_r = a1.tile([128, CSL], BF16, tag=f"sr{g}",
                                          bufs=2, name=f"sr{g}")
                            nc.vector.tensor_add(s_r, m1, m2)
                            s_i = a1.tile([128, CSL], BF16, tag=f"si{g}",
                                          bufs=2, name=f"si{g}")
                            nc.vector.tensor_sub(s_i, m3, m4)
                            s_loc[g] = (s_r, s_i)

                        for g in range(NT):
                            cc = a1.tile([128, CSL], BF16, tag=f"cc{g}",
                                         bufs=2, name="cc")
                            nc.vector.tensor_mul(cc, g_rep[g],
                                                 cosb_sb[:, sl])
                            C = a1.tile([128, CSL], BF16, tag=f"C{g}",
                                        bufs=2, name="C")
                            nc.vector.tensor_tensor_scan(
                                C, _free_bcast(rho_sb[:, g:g + 1], CSL), cc,
                                initial=(init_sb[:, 2 * g:2 * g + 1]
                                         if cs == 0
                                         else C_t[g][:, CSL - 1:CSL]),
                                op0=AOT.mult, op1=AOT.add,
                            )
                            S = a1.tile([128, CSL], BF16, tag=f"S{g}",
                                        bufs=2, name="S")
                            nc.vector.tensor_tensor_scan(
                                S, _free_bcast(rho_sb[:, g:g + 1], CSL),
                                ss_t[g],
                                initial=(init_sb[:, 2 * g + 1:2 * g + 2]
                                         if cs == 0
                                         else S_t[g][:, CSL - 1:CSL]),
                                op0=AOT.mult, op1=AOT.add,
                            )
                            newC[g], newS[g] = C, S
                            if g > 0:
                                rotate_back(g - 1)
                                finish_tile(g - 1)
                        rotate_back(NT - 1)
                        finish_tile(NT - 1)
                        C_t, S_t = newC, newS
                        return s_loc

                    def emit_a2(cs, s_loc):
                        grp = next(gi for gi, (lo, hi) in enumerate(GRP_CS)
                                   if lo <= cs < hi)
                        zm_st = a1.tile([128, 4, OUT], BF16, tag="zm_st",
                                        bufs=2, name="zm_st")
                        for w0 in range(0, CSL // 128, WAVE):
                            pss = [psum.tile([128, OUT], FP32, tag="zm",
                                             bufs=4, name="ps_zm")
                                   for _ in range(WAVE)]
                            for wi in range(WAVE):
                                nc.scalar.copy(pss[wi], bmixb_sb)
                            for g in range(NT):
                                for fld in range(2):
                                    k = 2 * g + fld
                                    for wi in range(WAVE):
                                        tch = w0 + wi
                                        nc.tensor.matmul(
                                            pss[wi],
                                            s_loc[g][fld][
                                                :, tch * 128:(tch + 1) * 128],
                                            wmix_sb[:, k, :],
                                            start=False,
                                            stop=(k == KCH - 1),
                                        )
                            for wi in range(WAVE):
                                nc.scalar.copy(zm_st[:, w0 + wi, :], pss[wi])
                        row0 = cs * CSL - SLS[grp][0]
                        nc.scalar.dma_start(
                            bass.AP(tensor=zm_d[grp].tensor,
                                    offset=zm_d[grp].offset + row0 * OUT,
                                    ap=[[OUT, 128], [128 * OUT, 4],
                                        [1, OUT]]),
                            zm_st,
                        )

                    # ---- fused pipeline ---------------------------------
                    emit_a0(0)
                    emit_repl(0)
                    for cs in range(NCS):
                        if cs + 1 < NCS:
                            emit_a0(cs + 1)
                        s_loc = emit_ew(cs)
                        if cs + 1 < NCS:
                            emit_repl(cs + 1)
                        if cs == BPREP_POS:
                            emit_bprep()
                        for grp, pos in RS_POS.items():
                            if pos == cs:
                                emit_rs(grp)
                        emit_a2(cs, s_loc)
                    for grp, pos in RS_POS.items():
                        if pos >= NCS:
                            emit_rs(grp)

                    # ---- phase B: LN + skip, 4 row blocks ---------------
                    res_t = pb.tile([128, 4, OUT], FP32, tag="res",
                                    name="res")
                    for bi in B_ORDER:
                        parts = B_BLOCKS[bi]
                        zm_sb = pb.tile([128, OUT], BF16, tag="zm_sb",
                                        name="zm_sb")
                        p0 = 0
                        for (grp, off, n) in parts:
                            nc.sync.dma_start(
                                zm_sb[p0:p0 + n, :],
                                zm_own_d[grp][off:off + n, :],
                            )
                            p0 += n
                        kt = bi
                        v = pb.tile([128, OUT], BF16, tag="v", name="v")
                        nc.gpsimd.tensor_mul(v, zm_sb, gout_st[:, kt, :])
                        stats = pb.tile([128, 6], FP32, tag="stats",
                                        name="stats")
                        nc.vector.bn_stats(stats, v)
                        mv = pb.tile([128, 2], FP32, tag="mv", name="mv")
                        nc.vector.bn_aggr(mv, stats)
                        sd = pb.tile([128, 1], FP32, tag="sd", name="sd")
                        nc.scalar.activation(sd, mv[:, 1:2], AFT.Sqrt,
                                             bias=eps_sb)
                        rstd = pb.tile([128, 1], FP32, tag="rstd",
                                       name="rstd")
                        nc.vector.reciprocal(rstd, sd)
                        ln = pb.tile([128, OUT], BF16, tag="ln", name="ln")
                        nc.vector.tensor_scalar(
                            ln, v, mv[:, 0:1], rstd,
                            op0=AOT.subtract, op1=AOT.mult,
                        )
                        t2 = pb.tile([128, OUT], BF16, tag="t2", name="t2")
                        nc.vector.scalar_tensor_tensor(
                            t2, gout_st[:, kt, :], 1.0, skip_st[:, kt, :],
                            op0=AOT.subtract, op1=AOT.mult,
                        )
                        nc.gpsimd.tensor_sub(res_t[:, bi, :], ln, t2)
                        nc.sync.dma_start(
                            outc[bi * 128:(bi + 1) * 128, :],
                            res_t[:, bi, :],
                        )

    nc.compile()
    return nc


def _xb_rows(c):
    """Global x-row indices, in xb/outc order, for core c (4 x 128)."""
    rows = []
    starts = [SLS[g][0] + c * BLS[g] for g in range(3)]
    for (grp, off, n) in [p for blk in B_BLOCKS for p in blk]:
        pass
    for blk in B_BLOCKS:
        for (grp, off, n) in blk:
            rows.extend(range(starts[grp] + off, starts[grp] + off + n))
    return np.array(rows)


def _prep_inputs(inputs):
    """Host-side: slice/rearrange FULL inputs into 8 per-core input maps."""
    x = np.asarray(inputs["x"], np.float32)
    state0 = np.asarray(inputs["state0"], np.float32)  # (1, TR, CTX, 2)
    a = np.abs(np.asarray(inputs["ffa_a"], np.float64))
    b = np.asarray(inputs["ffa_b"], np.float64)
    W_pre = np.asarray(inputs["W_pre"], np.float32)
    b_pre = np.asarray(inputs["b_pre"], np.float32)
    W_gin = np.asarray(inputs["W_gin"], np.float32)
    b_gin = np.asarray(inputs["b_gin"], np.float32)
    W_gout = np.asarray(inputs["W_gout"], np.float32)
    b_gout = np.asarray(inputs["b_gout"], np.float32)
    W_skip = np.asarray(inputs["W_skip"], np.float32)
    b_skip = np.asarray(inputs["b_skip"], np.float32)
    W_mix = np.asarray(inputs["W_mix"], np.float32)
    b_mix = np.asarray(inputs["b_mix"], np.float32)

    bf16 = mybir.dt.np(BF16)

    t_idx = np.arange(T, dtype=np.float64)
    ang = b[:, None] * t_idx[None, :]              # (CTX, T)
    cosb = np.tile(np.cos(ang), (2, 1)).astype(bf16)     # (128, T)
    sinb = np.tile(np.sin(ang), (2, 1)).astype(bf16)
    cossin = np.concatenate([cosb, sinb], axis=1)        # (128, 2T)
    prepl = np.zeros((NT, 8, 128), bf16)           # replication patterns
    for g in range(NT):
        prepl[g, 2 * g, 0:64] = 1.0
        prepl[g, 2 * g + 1, 64:128] = 1.0

    rho_v = np.exp(-a).astype(np.float32)          # (TR,)

    # scan initials from state0: R_{-1} = e^{i b_j} * s0 ; C init = Re,
    # S init = -Im (S-scan accumulates +sin terms, R_i = -S).
    s0r = state0[0, :, :, 0].astype(np.float64)    # (TR, CTX)
    s0i = state0[0, :, :, 1].astype(np.float64)
    cb1 = np.cos(b)[None, :]
    sb1 = np.sin(b)[None, :]
    initC = cb1 * s0r - sb1 * s0i                  # (TR, CTX)
    initS = -(sb1 * s0r + cb1 * s0i)

    # W_mix rows: row(i, j, re/im) = i*128 + fld*64 + j
    Wm = W_mix.reshape(TR, 2, CTX, OUT)            # [i][fld][j][o]

    xTb = np.ascontiguousarray(x.T.astype(bf16))   # (IN, T), same all cores
    wgout = W_gout.reshape(4, 128, OUT).astype(bf16)
    wskip = W_skip.reshape(4, 128, OUT).astype(bf16)
    ones_row = np.ones((1, 128), bf16)

    in_maps = []
    for c in range(NCORES):
        rho = np.empty((128, NT), np.float32)
        init_cs = np.empty((128, 2 * NT), np.float32)
        wmix = np.empty((KCH, 128, OUT), bf16)
        for g in range(NT):
            for il in range(2):
                tr = 8 * c + 2 * g + il
                sl = slice(il * 64, (il + 1) * 64)
                rho[sl, g] = rho_v[tr]
                init_cs[sl, 2 * g] = initC[tr]
                init_cs[sl, 2 * g + 1] = initS[tr]
                wmix[2 * g, sl] = Wm[tr, 0].astype(bf16)
                wmix[2 * g + 1, sl] = Wm[tr, 1].astype(bf16)
        trs = slice(8 * c, 8 * c + 8)
        Wpg = np.zeros((IN, 64), np.float32)
        Wpg[:, 0:TPC] = W_pre[:, trs]
        Wpg[:, 32:32 + TPC] = W_gin[:, trs]
        bias_pg_full = np.zeros((64, 1), np.float32)
        bias_pg_full[0:TPC, 0] = b_pre[trs]
        bias_pg_full[32:32 + TPC, 0] = b_gin[trs]
        xb = x[_xb_rows(c)]                              # (TL, IN)
        bmixb = np.broadcast_to(
            (b_mix if c == 0 else np.zeros_like(b_mix))[None, :],
            (128, OUT)).astype(np.float32).copy()
        in_maps.append({
            "xT": xTb,
            "xbT": np.ascontiguousarray(xb.T.astype(bf16)),
            "wpg": Wpg.reshape(4, 128, 64).astype(bf16),
            "bias_pg": bias_pg_full,
            "cossin": cossin,
            "rho": rho,
            "init_cs": init_cs,
            "prepl": prepl,
            "wmix": wmix,
            "bmixb": bmixb,
            "wgout": wgout,
            "wskip": wskip,
            "bgout": b_gout[None, :].astype(bf16),
            "bskip": b_skip[None, :].astype(bf16),
            "ones_row": ones_row,
        })
    return in_maps


def _assemble(results) -> np.ndarray:
    """Scatter per-core outc rows back to their global x-row positions."""
    out = np.empty((T, OUT), np.float32)
    for c in range(NCORES):
        oc = np.asarray(results[c]["outc"])
        out[_xb_rows(c)] = oc
    return out


def _get_module(reps: int = 1):
    key = f"nc{reps}"
    if key not in _CACHE:
        _CACHE[key] = _build_module(reps)
    return _CACHE[key]


def kernel(**inputs) -> np.ndarray:
    nc = _get_module()
    in_maps = _prep_inputs(inputs)
    res = run_bass_kernel_spmd(nc, in_maps, list(range(NCORES)))
    return _assemble(res.results)


if __name__ == "__main__":
    import reference  # only available when run inside /root/problem
    inputs = reference.setup_inputs()
    out = kernel(**{k: np.asarray(v) for k, v in inputs.items()})
    print("kernel output", out.shape, out.dtype)
